# revision 6
# baseline (speedup 1.0000x reference)
"""Bahdanau (additive) attention kernel for Trainium2, 8 NeuronCores.

Problem shapes: inp (B=4, T=128, D=512), context (B=4, S=512, D=512).
  wq   = inp @ Wq.T + bq                      (B,T,D)
  uh   = context @ Wc.T                       (B,S,D)
  align= einsum('btsd,d->bts', tanh(wq[:,:,None,:]+uh[:,None,:,:]), v)
  a    = softmax(align, -1)                   (B,T,S)
  c    = einsum('bts,bsd->btd', a, context)
  attn = concat([c, inp], -1) @ Wout.T + bout (B,T,D)
Returns (attn, a).

Sharding: 8 cores, core c handles batch b=c//2 and target-half th=c%2
(64 target positions per core). Weights replicated.

The dominant cost is the elementwise tanh over 16.8M elements per core.
This build splits that work across THREE engines per (t, d-chunk) unit
of [128 x 512]:
  - ACT: exact tanh on pre-summed tiles (adds from Pool or DVE)
  - Pool (gpsimd): tensor_scalar pre-adds for part of the ACT share
  - DVE: a clamped degree-7 odd-polynomial tanh evaluated by ONE custom
    DVE uop program (registered at import time): units get
    tensor_scalar (add bias, min B) -> tensor_scalar (max -B) ->
    custom op  g = (((c7 t + c5) t + c3) t + c1) * xc,  t = xc^2,
    which writes tanh-valued fp16 directly (max abs err ~8e-3 on the
    DVE share only; error budget checked end-to-end).
The per-t v-weighted d-reduction stays on the PE via one-hot shifted-Z
matmuls; softmax and the output projection are unchanged from the
ACT-only build.
"""

import numpy as np

import concourse.bacc as bacc
import concourse.tile as tile
from concourse import mybir
from concourse.bass import ds, ts
from concourse.bass_utils import run_bass_kernel_spmd
from concourse.masks import make_identity

F32 = mybir.dt.float32
F16 = mybir.dt.float16

B, T, S, D = 4, 128, 512, 512
N_CORES = 8
TH = T // 2  # 64 target positions per core
NCH = D // 128  # 4 partition chunks of the model dim
TBLK = 4  # target positions per main-loop block
NBLK = TH // TBLK

# ---- custom DVE op registration (degree-7 / degree-13 tanh) ----------------
from concourse import dve_ops as _dvo
from concourse.dve_spec import (
    Spec, Src0, Src1, C0, C1, C2, Latch, _has_src1, lower,
)
from concourse.dve_uop import DveOpSpec

# deg-7 odd minimax of tanh on [-B7, B7] with clamped tails
C7_, C5_, C3_, C1_ = (-0.002195027395932815, 0.0360726378920793,
                      -0.23082162587592386, 0.9654708772342129)
B7 = 2.40
# deg-13 (op pair) coefficients
P13, Q13, R13, W13 = (-41.46353605682129, 704.1415577602208,
                      -6400.397209947969, 34799.70434428201)
S13, U13, A13 = -126040.35564664593, 418583.06611382164, 2.3717188914682717e-06
B13 = 3.30


def _register(name, spec, subdim=False):
    for o in _dvo.OPS:
        if o.name == name:
            return o
    row = _dvo._CUSTOM_DVE_ROW_BASE + len(_dvo.OPS)
    assert row < 0x20
    _dvo._SUB_OPCODE_FOR_NAME[name] = row
    uops = lower(spec, ver="v3")
    sha = DveOpSpec(name=name, opcode=row, uops=uops,
                    rd1_en=_has_src1(spec)).sha("v3")
    op = _dvo.DveOp(name, spec, subdim=subdim, uops_sha={"v3": sha})
    _dvo.OPS.append(op)
    _dvo.CUSTOM_DVE_SPECS[name] = spec
    return op


def _tanh7_ref(in0, in1, s0, s1, imm2):
    xc = in0.astype(np.float32)
    t = xc * xc
    return ((s0 * t + s1) * t + imm2) * t * xc + in1 * xc


def _p13a_ref(in0, in1, s0, s1, imm2):
    t = in0.astype(np.float32) ** 2
    return (((t + s0) * t + s1) * t + imm2) * t + in1


def _p13b_ref(in0, in1, s0, s1, imm2):
    x1 = in1.astype(np.float32)
    t = x1 * x1
    return ((in0 * t + s0) * t + s1) * imm2 * x1


_t = Src0 * Src0
TANH7 = _register(
    "TANH7_ANT",
    Spec(body=(((C0 * _t + C1) * _t + C2) * _t + Latch(Src1)) * Src0,
         reference=_tanh7_ref),
)
TANH13A = _register(
    "TANH13A_ANT",
    Spec(body=(((_t + C0) * _t + C1) * _t + C2) * _t + Latch(Src1),
         reference=_p13a_ref),
)
_t2 = Src1 * Src1
TANH13B = _register(
    "TANH13B_ANT",
    Spec(body=((Src0 * _t2 + C0) * _t2 + C1) * C2 * Src1,
         reference=_p13b_ref),
)

# ---- per-block unit schedule ----------------------------------------------
# unit u = k*TBLK + tl (k-major). Tail units of each block go to the DVE
# polynomial stream; the rest are exact-ACT with pre-adds split Pool/DVE.
N_D7 = 5       # DVE deg-7 units per block (tail units)
N_D13 = 0      # DVE deg-13 units per block (before the deg-7 ones)
N_POOL = 6     # leading ACT units whose pre-add runs on Pool

_NC_CACHE = {}


def _build_nc():
    nc = bacc.Bacc("TRN2", target_bir_lowering=False, debug=False, num_devices=N_CORES)

    inpT = nc.dram_tensor("inpT", [D, TH], F16, kind="ExternalInput")
    ctxT = nc.dram_tensor("ctxT", [D, S], F16, kind="ExternalInput")
    wqT = nc.dram_tensor("wqT", [D, D], F16, kind="ExternalInput")
    wcT = nc.dram_tensor("wcT", [D, D], F16, kind="ExternalInput")
    woutT = nc.dram_tensor("woutT", [2 * D, D], F16, kind="ExternalInput")
    bq = nc.dram_tensor("bq", [D], F32, kind="ExternalInput")
    v = nc.dram_tensor("v", [D], F32, kind="ExternalInput")
    bout = nc.dram_tensor("bout", [D], F32, kind="ExternalInput")
    attn = nc.dram_tensor("attn", [TH, D], F32, kind="ExternalOutput")
    align = nc.dram_tensor("align", [TH, S], F32, kind="ExternalOutput")

    with tile.TileContext(nc) as tc:
        _emit(nc, tc, inpT, ctxT, wqT, wcT, woutT, bq, v, bout, attn, align)
    nc.compile()
    return nc


def _emit(nc, tc, inpT, ctxT, wqT, wcT, woutT, bq, v, bout, attn, align):
    Tanh = mybir.ActivationFunctionType.Tanh
    Exp = mybir.ActivationFunctionType.Exp
    Add = mybir.AluOpType.add
    Min = mybir.AluOpType.min
    Max = mybir.AluOpType.max
    ND = N_D7 + N_D13       # DVE units per block (tail)
    NA = 16 - ND            # ACT units per block
    NPOOL = min(N_POOL, NA)
    with (
        tc.tile_pool(name="persist", bufs=1) as P,
        tc.tile_pool(name="sums", bufs=3) as sums,
        tc.tile_pool(name="tanhs", bufs=3) as tanhs,
        tc.tile_pool(name="dvet", bufs=3) as dvet,
        tc.tile_pool(name="al_ps", bufs=1, space="PSUM") as al_ps,
        tc.tile_pool(name="mm_ps", bufs=2, space="PSUM") as mm_ps,
        tc.tile_pool(name="tr_ps", bufs=2, space="PSUM") as tr_ps,
        tc.tile_pool(name="o_ps", bufs=1, space="PSUM") as o_ps,
    ):
        # ---- persistent SBUF tiles + loads -------------------------------
        def load_wide(name, dram, engine=None):
            rows, F = dram.shape
            C = rows // 128
            t = P.tile([128, C * F], F16, name=name, tag=name)
            eng = engine or nc.sync
            eng.dma_start(
                out=t.rearrange("p (c f) -> p c f", c=C),
                in_=dram.ap().rearrange("(c p) f -> p c f", p=128),
            )
            return t

        ctxT_all = load_wide("ctxT_all", ctxT)
        wcT_all = P.tile([128, NCH * D], F16, name="wcT_all", tag="wcT_all")
        wcT_in3 = wcT.ap().rearrange("(c p) f -> p c f", p=128)
        wcT_out3 = wcT_all.rearrange("p (c f) -> p c f", c=NCH)
        wqT_all = P.tile([128, NCH * D], F16, name="wqT_all", tag="wqT_all")
        wqT_in3 = wqT.ap().rearrange("(c p) f -> p c f", p=128)
        wqT_out3 = wqT_all.rearrange("p (c f) -> p c f", c=NCH)
        nc.scalar.dma_start(out=wcT_out3[:, :, 0:256], in_=wcT_in3[:, :, 0:256])
        nc.scalar.dma_start(out=wqT_out3[:, :, 0:256], in_=wqT_in3[:, :, 0:256])
        inpT_all = load_wide("inpT_all", inpT)
        bq_sb = P.tile([128, NCH], F32, name="bq_sb", tag="bq_sb")
        nc.sync.dma_start(out=bq_sb, in_=bq.ap().rearrange("(k p) -> p k", p=128))
        v_sb = P.tile([128, NCH], F32, name="v_sb", tag="v_sb")
        nc.sync.dma_start(out=v_sb, in_=v.ap().rearrange("(k p) -> p k", p=128))
        nc.scalar.dma_start(out=wcT_out3[:, :, 256:512], in_=wcT_in3[:, :, 256:512])
        nc.scalar.dma_start(out=wqT_out3[:, :, 256:512], in_=wqT_in3[:, :, 256:512])
        ctxT_sb = [ctxT_all[:, ds(S * i, S)] for i in range(NCH)]
        wcT_sb = [wcT_all[:, ds(D * i, D)] for i in range(NCH)]
        wqT_sb = [wqT_all[:, ds(D * i, D)] for i in range(NCH)]
        inpT_sb = [inpT_all[:, ds(TH * i, TH)] for i in range(NCH)]

        # PE warmup: ramps the continuous-busy clock before real matmuls.
        warm_sb = P.tile([128, S], F16, name="warm_sb", tag="warm_sb")
        nc.vector.memset(warm_sb, 0.0)
        warm_ps = mm_ps.tile([128, S], F32, name="warm_ps", tag="uh_ps")
        for r in range(8):
            nc.tensor.matmul(warm_ps[0:64, :], lhsT=warm_sb[:, 0:64], rhs=warm_sb,
                             start=(r == 0), stop=(r == 7))

        # constants for the DVE polynomial stream
        c1v = P.tile([128, 1], F32, name="c1v", tag="c1v")
        nc.gpsimd.memset(c1v, float(C1_))
        w13v = P.tile([128, 1], F32, name="w13v", tag="w13v")
        nc.gpsimd.memset(w13v, float(W13))

        # Z[k]: zeros with v chunk k at column 63 (shifted-window one-hot)
        Z = []
        for k in range(NCH):
            z = P.tile([128, 2 * TH - 1], F16, name=f"Z{k}", tag=f"Z{k}")
            nc.vector.memset(z, 0.0)
            Z.append(z)

        ident = P.tile([128, 128], F16, name="ident", tag="ident")
        make_identity(nc, ident)
        ones_sb = P.tile([1, TH], F16, name="ones_sb", tag="ones_sb")
        nc.vector.memset(ones_sb, 1.0)

        def load_epilogue_tensors():
            woutT_all = load_wide("woutT_all", woutT, nc.scalar)
            woutT_sb = [woutT_all[:, ds(D * i, D)] for i in range(2 * NCH)]
            bout_f32 = P.tile([1, D], F32, name="bout_f32", tag="bout_f32")
            nc.sync.dma_start(
                out=bout_f32, in_=bout.ap().rearrange("(o f) -> o f", o=1)
            )
            bout_sb = P.tile([1, D], F16, name="bout_sb", tag="bout_sb")
            nc.vector.tensor_copy(bout_sb, bout_f32)
            return woutT_sb, bout_sb

        # ---- uh^T[e,s] = Wc @ context^T and wqb^T[e,t] = Wq @ inp^T + bq -
        uh_sb = [None] * NCH
        wqb_sb = [None] * NCH

        def prologue_phase(ks):
            for k in ks:
                ps = mm_ps.tile([128, S], F32, name="uh_ps", tag="uh_ps")
                for j in range(NCH):
                    nc.tensor.matmul(
                        ps,
                        lhsT=wcT_sb[j][:, ts(k, 128)],
                        rhs=ctxT_sb[j],
                        start=(j == 0),
                        stop=(j == NCH - 1),
                    )
                wps = tr_ps.tile([128, TH], F32, name="wq_ps", tag="wq_ps", bufs=1)
                for j in range(NCH):
                    nc.tensor.matmul(
                        wps,
                        lhsT=wqT_sb[j][:, ts(k, 128)],
                        rhs=inpT_sb[j],
                        start=(j == 0),
                        stop=(j == NCH - 1),
                    )
                u = P.tile([128, S], F16, name=f"uh{k}", tag=f"uh{k}")
                nc.vector.tensor_copy(u, ps)
                uh_sb[k] = u
                w = P.tile([128, TH], F32, name=f"wqb{k}", tag=f"wqb{k}")
                nc.vector.tensor_scalar_add(w, wps, bq_sb[:, k : k + 1])
                wqb_sb[k] = w

        prologue_phase([0])

        v16 = P.tile([128, NCH], F16, name="v16", tag="v16")
        nc.vector.tensor_copy(v16, v_sb)
        for k in range(NCH):
            nc.vector.tensor_copy(Z[k][:, TH - 1 : TH], v16[:, k : k + 1])

        # ---- main loop ---------------------------------------------------
        HT = TH // 2  # 32 rows per align half
        al_half = [
            al_ps.tile([HT, S], F32, name=f"al{h}", tag=f"al{h}") for h in range(2)
        ]
        woutT_sb = bout_sb = None
        mm_count = [0, 0]  # emitted Z-matmuls per half
        MM_TOTAL = 8 * 16  # per half

        def epilogue_half(h2, woutT_sb, bout_sb):
            rows = ds(h2 * HT, HT)
            p_h = P.tile([HT, S], F32, name=f"p{h2}", tag=f"p{h2}")
            ssum = P.tile([HT, 1], F32, name=f"ssum{h2}", tag=f"ssum{h2}")
            if h2 == 0:
                nc.scalar.activation(p_h, al_half[h2], Exp)
                nc.vector.reduce_sum(ssum, p_h, axis=mybir.AxisListType.X)
            else:
                nc.scalar.activation(
                    p_h, al_half[h2], Exp, accum_out=ssum[:, 0:1]
                )
            rcp = P.tile([HT, 1], F32, name=f"rcp{h2}", tag=f"rcp{h2}")
            nc.vector.reciprocal(rcp, ssum)
            a16 = P.tile([HT, S], F16, name=f"a16_{h2}", tag=f"a16_{h2}")
            nc.vector.tensor_scalar_mul(a16, p_h, rcp[:, 0:1])
            nc.vector.tensor_scalar_mul(align_sb[rows, :], p_h, rcp[:, 0:1])
            nc.sync.dma_start(out=align.ap()[rows, :], in_=align_sb[rows, :])

            alT_ps = tr_ps.tile(
                [128, NCH * HT], F16, name="alT_ps", tag="alT_ps", bufs=1
            )
            for i in range(NCH):
                nc.tensor.transpose(
                    alT_ps[:, ts(i, HT)], a16[:, ts(i, 128)], ident[0:HT, 0:HT]
                )
            alT = P.tile([128, NCH * HT], F16, name=f"alT{h2}", tag=f"alT{h2}")
            nc.vector.tensor_copy(alT, alT_ps)

            out_ps = out_ps_h[h2]
            for sc in range(NCH):
                nc.tensor.matmul(
                    out_ps,
                    lhsT=alT[:, ts(sc, HT)],
                    rhs=M_sb[sc],
                    start=False,
                    stop=(sc == NCH - 1),
                )
            for eh in range(2):
                ecols = ds(eh * (D // 2), D // 2)
                nc.vector.tensor_copy(attn_sb[rows, ecols], out_ps[:, ecols])
                nc.sync.dma_start(
                    out=attn.ap()[rows, ecols], in_=attn_sb[rows, ecols]
                )

        out_ps_h = {}
        M_sb = [None] * NCH

        def emit_M_chunk(sc, woutT_sb):
            ps = mm_ps.tile([128, S], F32, name="M_ps", tag="uh_ps")
            for j in range(NCH):
                nc.tensor.matmul(
                    ps,
                    lhsT=ctxT_all[:, ds(S * j + 128 * sc, 128)],
                    rhs=woutT_sb[j],
                    start=(j == 0),
                    stop=(j == NCH - 1),
                )
            m = P.tile([128, S], F16, name=f"M{sc}", tag=f"M{sc}")
            nc.vector.tensor_copy(m, ps)
            M_sb[sc] = m

        def out_early(h2, woutT_sb, bout_sb):
            rows = ds(h2 * HT, HT)
            out_ps = o_ps.tile([HT, D], F32, name="out_ps", tag="out_ps", bufs=1)
            nc.tensor.matmul(
                out_ps, lhsT=ones_sb[:, 0:HT], rhs=bout_sb, start=True, stop=False
            )
            for f in range(NCH, 2 * NCH):
                nc.tensor.matmul(
                    out_ps,
                    lhsT=inpT_sb[f - NCH][:, rows],
                    rhs=woutT_sb[f],
                    start=False,
                    stop=False,
                )
            out_ps_h[h2] = out_ps

        def zmm(h2, k, t_loc, rhs_slice):
            # one Z-matmul accumulating row t_loc of al_half[h2]
            nc.tensor.matmul(
                al_half[h2],
                lhsT=Z[k][:, TH - 1 - t_loc : TH - 1 - t_loc + HT],
                rhs=rhs_slice,
                start=(mm_count[h2] == 0),
                stop=(mm_count[h2] == MM_TOTAL - 1),
            )
            mm_count[h2] += 1

        align_sb = P.tile([TH, S], F32, name="align_sb", tag="align_sb")
        attn_sb = P.tile([TH, D], F32, name="attn_sb", tag="attn_sb")
        HB = NBLK // 2  # main-loop blocks per align half

        def emit_block(tb):
            h2 = tb // HB
            jit = tb == 0
            # tiles for this block
            sum_t = sums.tile([128, NA * S], F16, name="sum_t", tag="sum_t")
            tanh_t = tanhs.tile([128, NA * S], F16, name="tanh_t", tag="tanh_t")
            xm_t = dvet.tile([128, ND * S], F16, name="xm_t", tag="xm_t")
            xc_t = dvet.tile([128, ND * S], F16, name="xc_t", tag="xc_t")
            g_t = dvet.tile([128, ND * S], F16, name="g_t", tag="g_t")
            if N_D13:
                p4_t = dvet.tile([128, N_D13 * S], F32, name="p4_t", tag="p4_t")

            def unit_add(u, ui, eng, Bclamp=None):
                # pre-add for unit u; ACT units (ui = index into sum_t),
                # DVE units (ui = index into xm_t, with min clamp)
                k, tl = divmod(u, TBLK)
                t = tb * TBLK + tl
                if Bclamp is None:
                    eng.tensor_scalar(
                        out=sum_t[:, ds(ui * S, S)], in0=uh_sb[k],
                        scalar1=wqb_sb[k][:, t : t + 1], scalar2=None, op0=Add,
                    )
                else:
                    eng.tensor_scalar(
                        out=xm_t[:, ds(ui * S, S)], in0=uh_sb[k],
                        scalar1=wqb_sb[k][:, t : t + 1], scalar2=Bclamp,
                        op0=Add, op1=Min,
                    )

            def dve_chain():
                # TS2 + custom ops over the packed DVE units
                if N_D13:
                    sl13 = ds(0, N_D13 * S)
                    nc.vector.tensor_scalar(
                        out=xc_t[:, sl13], in0=xm_t[:, sl13],
                        scalar1=float(-B13), scalar2=None, op0=Max,
                    )
                if N_D7:
                    sl7 = ds(N_D13 * S, N_D7 * S)
                    nc.vector.tensor_scalar(
                        out=xc_t[:, sl7], in0=xm_t[:, sl7],
                        scalar1=float(-B7), scalar2=None, op0=Max,
                    )
                if N_D13:
                    sl = ds(0, N_D13 * S)
                    nc.vector._custom_dve(
                        TANH13A, out=p4_t, in0=xc_t[:, sl], in1=w13v,
                        s0=float(P13), s1=float(Q13), imm2=float(R13),
                    )
                    nc.vector._custom_dve(
                        TANH13B, out=g_t[:, sl], in0=p4_t, in1=xc_t[:, sl],
                        s0=float(S13), s1=float(U13), imm2=float(A13),
                    )
                if N_D7:
                    sl = ds(N_D13 * S, N_D7 * S)
                    nc.vector._custom_dve(
                        TANH7, out=g_t[:, sl], in0=xc_t[:, sl], in1=c1v,
                        s0=float(C7_), s1=float(C5_), imm2=float(C3_),
                    )

            def pe_units(units, tile_, base):
                for i, u in enumerate(units):
                    k, tl = divmod(u, TBLK)
                    t_loc = (tb % HB) * TBLK + tl
                    zmm(h2, k, t_loc, tile_[:, ds((base + i) * S, S)])

            if jit:
                # chunk-at-a-time with just-in-time prologue phases
                for k in range(NCH):
                    for tl in range(TBLK):
                        u = k * TBLK + tl
                        if u < NA:
                            eng = nc.gpsimd if u < NPOOL else nc.vector
                            unit_add(u, u, eng)
                        else:
                            unit_add(u, u - NA, nc.vector,
                                     Bclamp=B13 if u - NA < N_D13 else B7)
                    if k + 1 < NCH:
                        prologue_phase([k + 1])
                    # tanh + matmuls for this chunk's ACT units
                    ka = [u for u in range(k * TBLK, (k + 1) * TBLK) if u < NA]
                    if ka:
                        lo, hi = ka[0], ka[-1] + 1
                        nc.scalar.activation(
                            tanh_t[:, ds(lo * S, (hi - lo) * S)],
                            sum_t[:, ds(lo * S, (hi - lo) * S)], Tanh,
                        )
                        pe_units(ka, tanh_t, lo)
                dve_chain()
                pe_units(list(range(NA, 16)), g_t, 0)
                return

            # steady-state block
            for u in range(NPOOL):
                unit_add(u, u, nc.gpsimd)
            for u in range(NPOOL, NA):
                unit_add(u, u, nc.vector)
            for i, u in enumerate(range(NA, 16)):
                unit_add(u, i, nc.vector,
                         Bclamp=B13 if i < N_D13 else B7)
            # ACT: two instructions (Pool-fed then DVE-fed units)
            nc.scalar.activation(
                tanh_t[:, ds(0, NPOOL * S)], sum_t[:, ds(0, NPOOL * S)], Tanh
            )
            if NA > NPOOL:
                nc.scalar.activation(
                    tanh_t[:, ds(NPOOL * S, (NA - NPOOL) * S)],
                    sum_t[:, ds(NPOOL * S, (NA - NPOOL) * S)], Tanh,
                )
            dve_chain()
            pe_units(list(range(0, NA)), tanh_t, 0)
            pe_units(list(range(NA, 16)), g_t, 0)

        for tb in range(NBLK):
            emit_block(tb)
            if tb == 0:
                woutT_sb, bout_sb = load_epilogue_tensors()
            if 2 <= tb <= 5:
                emit_M_chunk(tb - 2, woutT_sb)
            if tb == HB - 4:
                out_early(0, woutT_sb, bout_sb)
            if tb == NBLK - 4:
                out_early(1, woutT_sb, bout_sb)
            if tb % HB == HB - 1:
                epilogue_half(tb // HB, woutT_sb, bout_sb)


def get_nc():
    if "nc" not in _NC_CACHE:
        _NC_CACHE["nc"] = _build_nc()
    return _NC_CACHE["nc"]


def make_in_maps(inp, context, Wq, bq, Wc, v, Wout, bout):
    inp = np.asarray(inp, np.float32)
    context = np.asarray(context, np.float32)
    Wq = np.asarray(Wq, np.float32)
    bq = np.asarray(bq, np.float32)
    Wc = np.asarray(Wc, np.float32)
    v = np.asarray(v, np.float32)
    Wout = np.asarray(Wout, np.float32)
    bout = np.asarray(bout, np.float32)

    wqT = np.ascontiguousarray(Wq.T).astype(np.float16)
    wcT = np.ascontiguousarray(Wc.T).astype(np.float16)
    woutT = np.ascontiguousarray(Wout.T).astype(np.float16)
    in_maps = []
    for c in range(N_CORES):
        b, th = divmod(c, 2)
        in_maps.append(
            {
                "inpT": np.ascontiguousarray(
                    inp[b, th * TH : (th + 1) * TH].T
                ).astype(np.float16),
                "ctxT": np.ascontiguousarray(context[b].T).astype(np.float16),
                "wqT": wqT,
                "wcT": wcT,
                "woutT": woutT,
                "bq": bq,
                "v": v,
                "bout": bout,
            }
        )
    return in_maps


def run_on_device(in_maps, **kwargs):
    nc = get_nc()
    return run_bass_kernel_spmd(nc, in_maps, core_ids=list(range(N_CORES)), **kwargs)


def kernel(inp, context, Wq, bq, Wc, v, Wout, bout):
    in_maps = make_in_maps(inp, context, Wq, bq, Wc, v, Wout, bout)
    res = run_on_device(in_maps)
    attn = np.empty((B, T, D), np.float32)
    align = np.empty((B, T, S), np.float32)
    for c in range(N_CORES):
        b, th = divmod(c, 2)
        attn[b, th * TH : (th + 1) * TH] = res.results[c]["attn"]
        align[b, th * TH : (th + 1) * TH] = res.results[c]["align"]
    return attn, align


# revision 34
# speedup vs baseline: 1.3060x; 1.3060x over previous
"""Bahdanau (additive) attention kernel for Trainium2, 8 NeuronCores.

Problem shapes: inp (B=4, T=128, D=512), context (B=4, S=512, D=512).
  wq   = inp @ Wq.T + bq                      (B,T,D)
  uh   = context @ Wc.T                       (B,S,D)
  align= einsum('btsd,d->bts', tanh(wq[:,:,None,:]+uh[:,None,:,:]), v)
  a    = softmax(align, -1)                   (B,T,S)
  c    = einsum('bts,bsd->btd', a, context)
  attn = concat([c, inp], -1) @ Wout.T + bout (B,T,D)
Returns (attn, a).

Sharding: 8 cores, core c handles batch b=c//2 and target-half th=c%2
(64 target positions per core). Weights replicated.

The dominant cost is the elementwise tanh over 16.8M elements per core.
This build splits that work across THREE engines per (t, d-chunk) unit
of [128 x 512]:
  - ACT: exact tanh on pre-summed tiles (adds from Pool or DVE)
  - Pool (gpsimd): tensor_scalar pre-adds for part of the ACT share
  - DVE: a clamped degree-7 odd-polynomial tanh evaluated by ONE custom
    DVE uop program (registered at import time): units get
    tensor_scalar (add bias, min B) -> tensor_scalar (max -B) ->
    custom op  g = (((c7 t + c5) t + c3) t + c1) * xc,  t = xc^2,
    which writes tanh-valued fp16 directly (max abs err ~8e-3 on the
    DVE share only; error budget checked end-to-end).
The per-t v-weighted d-reduction stays on the PE via one-hot shifted-Z
matmuls; softmax and the output projection are unchanged from the
ACT-only build.
"""

import numpy as np

import concourse.bacc as bacc
import concourse.tile as tile
from concourse import mybir
from concourse.bass import ds, ts
from concourse.bass_utils import run_bass_kernel_spmd
from concourse.masks import make_identity

F32 = mybir.dt.float32
F16 = mybir.dt.float16

B, T, S, D = 4, 128, 512, 512
N_CORES = 8
TH = T // 2  # 64 target positions per core
NCH = D // 128  # 4 partition chunks of the model dim
TBLK = 4  # target positions per main-loop block
NBLK = TH // TBLK

# ---- custom DVE op registration (degree-7 / degree-13 tanh) ----------------
from concourse import dve_ops as _dvo
from concourse.dve_spec import (
    Spec, Src0, Src1, C0, C1, C2, Latch, _has_src1, lower,
)
from concourse.dve_uop import DveOpSpec

# deg-7 odd minimax of tanh on [-B7, B7] with clamped tails
C7_, C5_, C3_, C1_ = (-0.002195027395932815, 0.0360726378920793,
                      -0.23082162587592386, 0.9654708772342129)
B7 = 2.40
# deg-13 (op pair) coefficients
P13, Q13, R13, W13 = (-41.46353605682129, 704.1415577602208,
                      -6400.397209947969, 34799.70434428201)
S13, U13, A13 = -126040.35564664593, 418583.06611382164, 2.3717188914682717e-06
B13 = 3.30


def _register(name, spec, subdim=False):
    for o in _dvo.OPS:
        if o.name == name:
            return o
    row = _dvo._CUSTOM_DVE_ROW_BASE + len(_dvo.OPS)
    assert row < 0x20
    _dvo._SUB_OPCODE_FOR_NAME[name] = row
    uops = lower(spec, ver="v3")
    sha = DveOpSpec(name=name, opcode=row, uops=uops,
                    rd1_en=_has_src1(spec)).sha("v3")
    op = _dvo.DveOp(name, spec, subdim=subdim, uops_sha={"v3": sha})
    _dvo.OPS.append(op)
    _dvo.CUSTOM_DVE_SPECS[name] = spec
    return op


def _tanh7_ref(in0, in1, s0, s1, imm2):
    xc = in0.astype(np.float32)
    t = xc * xc
    return ((s0 * t + s1) * t + imm2) * t * xc + in1 * xc


def _p13a_ref(in0, in1, s0, s1, imm2):
    t = in0.astype(np.float32) ** 2
    return (((t + s0) * t + s1) * t + imm2) * t + in1


def _p13b_ref(in0, in1, s0, s1, imm2):
    x1 = in1.astype(np.float32)
    t = x1 * x1
    return ((in0 * t + s0) * t + s1) * imm2 * x1


_t = Src0 * Src0
TANH7 = _register(
    "TANH7_ANT",
    Spec(body=(((C0 * _t + C1) * _t + C2) * _t + Latch(Src1)) * Src0,
         reference=_tanh7_ref),
)
TANH13A = _register(
    "TANH13A_ANT",
    Spec(body=(((_t + C0) * _t + C1) * _t + C2) * _t + Latch(Src1),
         reference=_p13a_ref),
)
_t2 = Src1 * Src1
TANH13B = _register(
    "TANH13B_ANT",
    Spec(body=((Src0 * _t2 + C0) * _t2 + C1) * C2 * Src1,
         reference=_p13b_ref),
)

# ---- per-block unit schedule ----------------------------------------------
# unit u = k*TBLK + tl (k-major). Tail units of each block go to the DVE
# polynomial stream; the rest are exact-ACT with pre-adds split Pool/DVE.
# The last block of each half is all-ACT so the softmax isn't gated on the
# (serial) DVE op chain; its DVE units are redistributed to mid blocks.
ND_TB = [5, 5, 4, 5, 5, 4, 5, 0,
         5, 5, 4, 5, 5, 4, 5, 0]   # DVE deg-7 units per block (tail units)
ND_MAX = max(ND_TB)
N_D13 = 0      # deg-13 units per block (before the deg-7 ones; 0 = disabled)
N_POOL = 6     # leading ACT units whose pre-add runs on Pool

_NC_CACHE = {}


def _build_nc():
    nc = bacc.Bacc("TRN2", target_bir_lowering=False, debug=False, num_devices=N_CORES)

    inpT = nc.dram_tensor("inpT", [D, TH], F16, kind="ExternalInput")
    ctxT = nc.dram_tensor("ctxT", [D, S], F16, kind="ExternalInput")
    wqT = nc.dram_tensor("wqT", [D, D], F16, kind="ExternalInput")
    wcT = nc.dram_tensor("wcT", [D, D], F16, kind="ExternalInput")
    woutT = nc.dram_tensor("woutT", [2 * D, D], F16, kind="ExternalInput")
    bq = nc.dram_tensor("bq", [D], F32, kind="ExternalInput")
    v = nc.dram_tensor("v", [D], F32, kind="ExternalInput")
    bout = nc.dram_tensor("bout", [D], F32, kind="ExternalInput")
    attn = nc.dram_tensor("attn", [TH, D], F32, kind="ExternalOutput")
    align = nc.dram_tensor("align", [TH, S], F32, kind="ExternalOutput")

    with tile.TileContext(nc) as tc:
        _emit(nc, tc, inpT, ctxT, wqT, wcT, woutT, bq, v, bout, attn, align)
    nc.compile()
    return nc


def _emit(nc, tc, inpT, ctxT, wqT, wcT, woutT, bq, v, bout, attn, align):
    Tanh = mybir.ActivationFunctionType.Tanh
    Exp = mybir.ActivationFunctionType.Exp
    Add = mybir.AluOpType.add
    Min = mybir.AluOpType.min
    Max = mybir.AluOpType.max
    NPOOL = N_POOL
    with (
        tc.tile_pool(name="persist", bufs=1) as P,
        tc.tile_pool(name="sums", bufs=3) as sums,
        tc.tile_pool(name="tanhs", bufs=3) as tanhs,
        tc.tile_pool(name="dvet", bufs=3) as dvet,
        tc.tile_pool(name="al_ps", bufs=1, space="PSUM") as al_ps,
        tc.tile_pool(name="mm_ps", bufs=2, space="PSUM") as mm_ps,
        tc.tile_pool(name="tr_ps", bufs=2, space="PSUM") as tr_ps,
        tc.tile_pool(name="o_ps", bufs=1, space="PSUM") as o_ps,
    ):
        # ---- persistent SBUF tiles + loads -------------------------------
        def load_wide(name, dram, engine=None):
            rows, F = dram.shape
            C = rows // 128
            t = P.tile([128, C * F], F16, name=name, tag=name)
            eng = engine or nc.sync
            eng.dma_start(
                out=t.rearrange("p (c f) -> p c f", c=C),
                in_=dram.ap().rearrange("(c p) f -> p c f", p=128),
            )
            return t

        ctxT_all = load_wide("ctxT_all", ctxT)
        wcT_all = P.tile([128, NCH * D], F16, name="wcT_all", tag="wcT_all")
        wcT_in3 = wcT.ap().rearrange("(c p) f -> p c f", p=128)
        wcT_out3 = wcT_all.rearrange("p (c f) -> p c f", c=NCH)
        wqT_all = P.tile([128, NCH * D], F16, name="wqT_all", tag="wqT_all")
        wqT_in3 = wqT.ap().rearrange("(c p) f -> p c f", p=128)
        wqT_out3 = wqT_all.rearrange("p (c f) -> p c f", c=NCH)
        # DMA order: k=0's weights (wcT cols 0:128 via h1, wqT cols 0:128)
        # first so prologue phase 0 starts ASAP; then the rest of wcT so
        # later uh chunks aren't stuck behind the full wqT transfer.
        nc.scalar.dma_start(out=wcT_out3[:, :, 0:256], in_=wcT_in3[:, :, 0:256])
        nc.scalar.dma_start(out=wqT_out3[:, :, 0:128], in_=wqT_in3[:, :, 0:128])
        inpT_all = load_wide("inpT_all", inpT)
        bq_sb = P.tile([128, NCH], F32, name="bq_sb", tag="bq_sb")
        nc.sync.dma_start(out=bq_sb, in_=bq.ap().rearrange("(k p) -> p k", p=128))
        v_sb = P.tile([128, NCH], F32, name="v_sb", tag="v_sb")
        nc.sync.dma_start(out=v_sb, in_=v.ap().rearrange("(k p) -> p k", p=128))
        nc.scalar.dma_start(out=wcT_out3[:, :, 256:512], in_=wcT_in3[:, :, 256:512])
        nc.scalar.dma_start(out=wqT_out3[:, :, 128:512], in_=wqT_in3[:, :, 128:512])
        ctxT_sb = [ctxT_all[:, ds(S * i, S)] for i in range(NCH)]
        wcT_sb = [wcT_all[:, ds(D * i, D)] for i in range(NCH)]
        wqT_sb = [wqT_all[:, ds(D * i, D)] for i in range(NCH)]
        inpT_sb = [inpT_all[:, ds(TH * i, TH)] for i in range(NCH)]

        # PE warmup: ramps the continuous-busy clock before real matmuls.
        warm_sb = P.tile([128, S], F16, name="warm_sb", tag="warm_sb")
        nc.vector.memset(warm_sb, 0.0)
        warm_ps = mm_ps.tile([128, S], F32, name="warm_ps", tag="uh_ps")
        for r in range(8):
            nc.tensor.matmul(warm_ps[0:64, :], lhsT=warm_sb[:, 0:64], rhs=warm_sb,
                             start=(r == 0), stop=(r == 7))

        # keep-warm: dep-free dummy matmuls fill PE idle gaps so the
        # continuous-busy clock stays at max rate (a cold restart doubles
        # every matmul's cycle time for 3us). They sit in the PE FIFO at
        # points where the PE would otherwise wait on a producer.
        hot_ps = o_ps.tile([64, 384], F32, name="hot_ps", tag="hot_ps", bufs=1)

        def keep_warm(n):
            for _ in range(n):
                nc.tensor.matmul(hot_ps, lhsT=warm_sb[:, 0:64],
                                 rhs=warm_sb[:, 0:384], start=True, stop=True)

        # constants for the DVE polynomial stream
        c1v = P.tile([128, 1], F32, name="c1v", tag="c1v")
        nc.gpsimd.memset(c1v, float(C1_))
        w13v = P.tile([128, 1], F32, name="w13v", tag="w13v")
        nc.gpsimd.memset(w13v, float(W13))

        # Z[k]: zeros with v chunk k at column 63 (shifted-window one-hot)
        Z = []
        for k in range(NCH):
            z = P.tile([128, 2 * TH - 1], F16, name=f"Z{k}", tag=f"Z{k}")
            nc.vector.memset(z, 0.0)
            Z.append(z)

        ident = P.tile([128, 128], F16, name="ident", tag="ident")
        make_identity(nc, ident)
        ones_sb = P.tile([1, TH], F16, name="ones_sb", tag="ones_sb")
        nc.vector.memset(ones_sb, 1.0)

        def load_epilogue_tensors():
            woutT_all = load_wide("woutT_all", woutT, nc.scalar)
            woutT_sb = [woutT_all[:, ds(D * i, D)] for i in range(2 * NCH)]
            bout_f32 = P.tile([1, D], F32, name="bout_f32", tag="bout_f32")
            nc.sync.dma_start(
                out=bout_f32, in_=bout.ap().rearrange("(o f) -> o f", o=1)
            )
            bout_sb = P.tile([1, D], F16, name="bout_sb", tag="bout_sb")
            nc.vector.tensor_copy(bout_sb, bout_f32)
            return woutT_sb, bout_sb

        # ---- uh^T[e,s] = Wc @ context^T and wqb^T[e,t] = Wq @ inp^T + bq -
        uh_sb = [None] * NCH
        wqb_sb = [None] * NCH

        def prologue_phase(ks):
            for k in ks:
                ps = mm_ps.tile([128, S], F32, name="uh_ps", tag="uh_ps")
                for j in range(NCH):
                    nc.tensor.matmul(
                        ps,
                        lhsT=wcT_sb[j][:, ts(k, 128)],
                        rhs=ctxT_sb[j],
                        start=(j == 0),
                        stop=(j == NCH - 1),
                    )
                wps = tr_ps.tile([128, TH], F32, name="wq_ps", tag="wq_ps", bufs=1)
                for j in range(NCH):
                    nc.tensor.matmul(
                        wps,
                        lhsT=wqT_sb[j][:, ts(k, 128)],
                        rhs=inpT_sb[j],
                        start=(j == 0),
                        stop=(j == NCH - 1),
                    )
                u = P.tile([128, S], F16, name=f"uh{k}", tag=f"uh{k}")
                nc.vector.tensor_copy(u, ps)
                uh_sb[k] = u
                w = P.tile([128, TH], F32, name=f"wqb{k}", tag=f"wqb{k}")
                nc.vector.tensor_scalar_add(w, wps, bq_sb[:, k : k + 1])
                wqb_sb[k] = w

        prologue_phase([0])

        v16 = P.tile([128, NCH], F16, name="v16", tag="v16")
        nc.vector.tensor_copy(v16, v_sb)
        for k in range(NCH):
            nc.vector.tensor_copy(Z[k][:, TH - 1 : TH], v16[:, k : k + 1])

        # ---- main loop ---------------------------------------------------
        HT = TH // 2  # 32 rows per align half
        al_half = [
            al_ps.tile([HT, S], F32, name=f"al{h}", tag=f"al{h}") for h in range(2)
        ]
        woutT_sb = bout_sb = None
        mm_count = [0, 0]  # emitted Z-matmuls per half
        MM_TOTAL = 8 * 16  # per half

        def epilogue_softmax(h2, kw):
            rows = ds(h2 * HT, HT)
            p_h = P.tile([HT, S], F32, name=f"p{h2}", tag=f"p{h2}")
            ssum = P.tile([HT, 1], F32, name=f"ssum{h2}", tag=f"ssum{h2}")
            nc.scalar.activation(p_h, al_half[h2], Exp, accum_out=ssum[:, 0:1])
            rcp = P.tile([HT, 1], F32, name=f"rcp{h2}", tag=f"rcp{h2}")
            nc.vector.reciprocal(rcp, ssum)
            # align first so its (long-latency) DMA overlaps the attn tail
            nc.vector.tensor_scalar_mul(align_sb[rows, :], p_h, rcp[:, 0:1])
            nc.sync.dma_start(out=align.ap()[rows, :], in_=align_sb[rows, :])
            a16 = P.tile([HT, S], F16, name=f"a16_{h2}", tag=f"a16_{h2}")
            nc.vector.tensor_scalar_mul(a16, p_h, rcp[:, 0:1])

            keep_warm(kw)
            alT_ps = tr_ps.tile(
                [128, NCH * HT], F16, name="alT_ps", tag="alT_ps", bufs=1
            )
            for i in range(NCH):
                nc.tensor.transpose(
                    alT_ps[:, ts(i, HT)], a16[:, ts(i, 128)], ident[0:HT, 0:HT]
                )
            alT = P.tile([128, NCH * HT], F16, name=f"alT{h2}", tag=f"alT{h2}")
            nc.vector.tensor_copy(alT, alT_ps)

            out_ps = out_ps_h[h2]
            for sc in range(NCH):
                nc.tensor.matmul(
                    out_ps,
                    lhsT=alT[:, ts(sc, HT)],
                    rhs=M_sb[sc],
                    start=False,
                    stop=(sc == NCH - 1),
                )

        def epilogue_attn(h2):
            # copies split ACT/DVE (both can read PSUM); DMAs on the scalar
            # queue so they don't serialize behind the align DMA on SP
            rows = ds(h2 * HT, HT)
            out_ps = out_ps_h[h2]
            for eh in range(2):
                ecols = ds(eh * (D // 2), D // 2)
                if eh == 0:
                    nc.scalar.copy(attn_sb[rows, ecols], out_ps[:, ecols])
                else:
                    nc.vector.tensor_copy(attn_sb[rows, ecols], out_ps[:, ecols])
                nc.scalar.dma_start(
                    out=attn.ap()[rows, ecols], in_=attn_sb[rows, ecols]
                )

        out_ps_h = {}
        M_sb = [None] * NCH

        def emit_M_chunk(sc, woutT_sb):
            ps = mm_ps.tile([128, S], F32, name="M_ps", tag="uh_ps")
            for j in range(NCH):
                nc.tensor.matmul(
                    ps,
                    lhsT=ctxT_all[:, ds(S * j + 128 * sc, 128)],
                    rhs=woutT_sb[j],
                    start=(j == 0),
                    stop=(j == NCH - 1),
                )
            m = P.tile([128, S], F16, name=f"M{sc}", tag=f"M{sc}")
            nc.vector.tensor_copy(m, ps)
            M_sb[sc] = m

        def out_early(h2, woutT_sb, bout_sb):
            rows = ds(h2 * HT, HT)
            out_ps = o_ps.tile([HT, D], F32, name="out_ps", tag="out_ps", bufs=1)
            nc.tensor.matmul(
                out_ps, lhsT=ones_sb[:, 0:HT], rhs=bout_sb, start=True, stop=False
            )
            for f in range(NCH, 2 * NCH):
                nc.tensor.matmul(
                    out_ps,
                    lhsT=inpT_sb[f - NCH][:, rows],
                    rhs=woutT_sb[f],
                    start=False,
                    stop=False,
                )
            out_ps_h[h2] = out_ps

        def zmm(h2, k, t_loc, rhs_slice):
            # one Z-matmul accumulating row t_loc of al_half[h2]
            nc.tensor.matmul(
                al_half[h2],
                lhsT=Z[k][:, TH - 1 - t_loc : TH - 1 - t_loc + HT],
                rhs=rhs_slice,
                start=(mm_count[h2] == 0),
                stop=(mm_count[h2] == MM_TOTAL - 1),
            )
            mm_count[h2] += 1

        align_sb = P.tile([TH, S], F32, name="align_sb", tag="align_sb")
        attn_sb = P.tile([TH, D], F32, name="attn_sb", tag="attn_sb")
        HB = NBLK // 2  # main-loop blocks per align half

        blk = {}  # tb -> dict of tiles

        def unit_add(tb, u, ui, eng, Bclamp=None):
            # pre-add for unit u; ACT units (ui = index into sum_t),
            # DVE units (ui = index into xm_t, with min clamp)
            b = blk[tb]
            k, tl = divmod(u, TBLK)
            t = tb * TBLK + tl
            if Bclamp is None:
                eng.tensor_scalar(
                    out=b["sum"][:, ds(ui * S, S)], in0=uh_sb[k],
                    scalar1=wqb_sb[k][:, t : t + 1], scalar2=None, op0=Add,
                )
            else:
                eng.tensor_scalar(
                    out=b["xm"][:, ds(ui * S, S)], in0=uh_sb[k],
                    scalar1=wqb_sb[k][:, t : t + 1], scalar2=Bclamp,
                    op0=Add, op1=Min,
                )

        def stage1(tb):
            # producers: tile allocation + all pre-adds / TS1s for block tb
            ND = ND_TB[tb]
            NA = 16 - ND
            b = blk[tb] = dict(
                sum=sums.tile([128, 16 * S], F16, name="sum_t", tag="sum_t"),
                tanh=tanhs.tile([128, 16 * S], F16, name="tanh_t", tag="tanh_t"),
            )
            if ND:
                b["xm"] = dvet.tile([128, ND_MAX * S], F16, name="xm_t", tag="xm_t")
                b["xc"] = dvet.tile([128, ND_MAX * S], F16, name="xc_t", tag="xc_t")
                b["g"] = dvet.tile([128, ND_MAX * S], F16, name="g_t", tag="g_t")
            if N_D13:
                b["p4"] = dvet.tile([128, N_D13 * S], F32, name="p4_t", tag="p4_t")
            for u in range(NPOOL):
                unit_add(tb, u, u, nc.gpsimd)
            for u in range(NPOOL, NA):
                unit_add(tb, u, u, nc.vector)
            for i, u in enumerate(range(NA, 16)):
                unit_add(tb, u, i, nc.vector, Bclamp=B13 if i < N_D13 else B7)

        def stage2(tb):
            # TS2 + custom ops over the packed DVE units of block tb
            ND = ND_TB[tb]
            N_D7 = ND - N_D13
            if not ND:
                return
            b = blk[tb]
            xm_t, xc_t, g_t = b["xm"], b["xc"], b["g"]
            if N_D13:
                sl = ds(0, N_D13 * S)
                nc.vector.tensor_scalar(
                    out=xc_t[:, sl], in0=xm_t[:, sl],
                    scalar1=float(-B13), scalar2=None, op0=Max,
                )
                nc.vector._custom_dve(
                    TANH13A, out=b["p4"], in0=xc_t[:, sl], in1=w13v,
                    s0=float(P13), s1=float(Q13), imm2=float(R13),
                )
                nc.vector._custom_dve(
                    TANH13B, out=g_t[:, sl], in0=b["p4"], in1=xc_t[:, sl],
                    s0=float(S13), s1=float(U13), imm2=float(A13),
                )
            if N_D7:
                sl = ds(N_D13 * S, N_D7 * S)
                nc.vector.tensor_scalar(
                    out=xc_t[:, sl], in0=xm_t[:, sl],
                    scalar1=float(-B7), scalar2=None, op0=Max,
                )
                nc.vector._custom_dve(
                    TANH7, out=g_t[:, sl], in0=xc_t[:, sl], in1=c1v,
                    s0=float(C7_), s1=float(C5_), imm2=float(C3_),
                )

        def act_stage(tb):
            # one tanh instruction per block: all producers ran >=1 block
            # ago (software pipelining), so the coarse dep costs nothing
            # and the per-instruction overhead is paid once.
            NA = 16 - ND_TB[tb]
            b = blk[tb]
            nc.scalar.activation(
                b["tanh"][:, ds(0, NA * S)], b["sum"][:, ds(0, NA * S)], Tanh
            )

        def pe_units(tb, units, tile_, base):
            h2 = tb // HB
            for i, u in enumerate(units):
                k, tl = divmod(u, TBLK)
                t_loc = (tb % HB) * TBLK + tl
                zmm(h2, k, t_loc, tile_[:, ds((base + i) * S, S)])

        def pe_stage(tb, kw=9):
            ND = ND_TB[tb]
            NA = 16 - ND
            b = blk[tb]
            pe_units(tb, list(range(0, NA)), b["tanh"], 0)
            if ND:
                pe_units(tb, list(range(NA, 16)), b["g"], 0)
            keep_warm(kw)
            del blk[tb]

        def emit_block0():
            # chunk-at-a-time with just-in-time prologue phases (all-ACT)
            ND = ND_TB[0]
            NA = 16 - ND
            b = blk[0] = dict(
                sum=sums.tile([128, 16 * S], F16, name="sum_t", tag="sum_t"),
                tanh=tanhs.tile([128, 16 * S], F16, name="tanh_t", tag="tanh_t"),
            )
            if ND:
                b["xm"] = dvet.tile([128, ND_MAX * S], F16, name="xm_t", tag="xm_t")
                b["xc"] = dvet.tile([128, ND_MAX * S], F16, name="xc_t", tag="xc_t")
                b["g"] = dvet.tile([128, ND_MAX * S], F16, name="g_t", tag="g_t")
            if N_D13:
                b["p4"] = dvet.tile([128, N_D13 * S], F32, name="p4_t", tag="p4_t")
            # all adds + remaining prologue phases first: the uh_k matmuls
            # must not sit behind Z-matmuls (which wait on ACT) in the PE
            # FIFO. Block-0 adds are all-DVE: Pool's first work is block 1,
            # so the scheduler can't starve block-0's tanh stream with it.
            for k in range(NCH):
                for tl in range(TBLK):
                    u = k * TBLK + tl
                    if u < NA:
                        unit_add(0, u, u, nc.vector)
                    else:
                        unit_add(0, u, u - NA, nc.vector,
                                 Bclamp=B13 if u - NA < N_D13 else B7)
                if k + 1 < NCH:
                    prologue_phase([k + 1])
            for k in range(NCH):
                ka = [u for u in range(k * TBLK, (k + 1) * TBLK) if u < NA]
                if ka:
                    lo, hi = ka[0], ka[-1] + 1
                    nc.scalar.activation(
                        b["tanh"][:, ds(lo * S, (hi - lo) * S)],
                        b["sum"][:, ds(lo * S, (hi - lo) * S)], Tanh,
                    )
                    pe_units(0, ka, b["tanh"], lo)
            keep_warm(10)

        def finish_block0():
            ND = ND_TB[0]
            NA = 16 - ND
            if ND:
                stage2(0)
                pe_units(0, list(range(NA, 16)), blk[0]["g"], 0)
            del blk[0]

        # software-pipelined main loop: block n's producers (stage1) are
        # emitted ~2 blocks ahead of its DVE op chain (stage2), so the ACT
        # stream is never queued behind the custom-op work on the DVE FIFO.
        emit_block0()
        woutT_sb, bout_sb = load_epilogue_tensors()
        stage1(1)
        finish_block0()

        def last_block(tb):
            # final block of the run: all-ACT; emit tanh+matmuls in 4-unit
            # quarters so the align accumulation finishes (and the epilogue
            # starts) right after the last quarter instead of after one
            # monolithic 16-unit activation.
            b = blk[tb]
            for q in range(4):
                qs = ds(q * 4 * S, 4 * S)
                nc.scalar.activation(b["tanh"][:, qs], b["sum"][:, qs], Tanh)
                pe_units(tb, list(range(q * 4, q * 4 + 4)), b["tanh"], q * 4)
            keep_warm(4)
            del blk[tb]

        for tb in range(1, NBLK):
            if tb == NBLK - 1:
                stage2(tb)
                last_block(tb)
                epilogue_softmax(1, kw=7)
                epilogue_attn(1)
                break
            stage2(tb)
            act_stage(tb)
            pe_stage(tb)
            if tb + 1 < NBLK and (tb + 1) not in blk:
                stage1(tb + 1)
            if tb + 2 < NBLK:
                stage1(tb + 2)
            if 2 <= tb <= 5:
                emit_M_chunk(tb - 2, woutT_sb)
            if tb == HB - 4:
                out_early(0, woutT_sb, bout_sb)
            if tb == NBLK - 4:
                out_early(1, woutT_sb, bout_sb)
            # half-0 epilogue is emitted one block late so its cross-engine
            # chain doesn't head-of-line-block the next block's producers
            if tb == HB:
                epilogue_softmax(0, kw=4)
            if tb == HB + 1:
                epilogue_attn(0)
            if tb == NBLK - 1:
                epilogue_softmax(1, kw=7)
                epilogue_attn(1)


def get_nc():
    if "nc" not in _NC_CACHE:
        _NC_CACHE["nc"] = _build_nc()
    return _NC_CACHE["nc"]


def make_in_maps(inp, context, Wq, bq, Wc, v, Wout, bout):
    inp = np.asarray(inp, np.float32)
    context = np.asarray(context, np.float32)
    Wq = np.asarray(Wq, np.float32)
    bq = np.asarray(bq, np.float32)
    Wc = np.asarray(Wc, np.float32)
    v = np.asarray(v, np.float32)
    Wout = np.asarray(Wout, np.float32)
    bout = np.asarray(bout, np.float32)

    wqT = np.ascontiguousarray(Wq.T).astype(np.float16)
    wcT = np.ascontiguousarray(Wc.T).astype(np.float16)
    woutT = np.ascontiguousarray(Wout.T).astype(np.float16)
    in_maps = []
    for c in range(N_CORES):
        b, th = divmod(c, 2)
        in_maps.append(
            {
                "inpT": np.ascontiguousarray(
                    inp[b, th * TH : (th + 1) * TH].T
                ).astype(np.float16),
                "ctxT": np.ascontiguousarray(context[b].T).astype(np.float16),
                "wqT": wqT,
                "wcT": wcT,
                "woutT": woutT,
                "bq": bq,
                "v": v,
                "bout": bout,
            }
        )
    return in_maps


def run_on_device(in_maps, **kwargs):
    nc = get_nc()
    return run_bass_kernel_spmd(nc, in_maps, core_ids=list(range(N_CORES)), **kwargs)


def kernel(inp, context, Wq, bq, Wc, v, Wout, bout):
    in_maps = make_in_maps(inp, context, Wq, bq, Wc, v, Wout, bout)
    res = run_on_device(in_maps)
    attn = np.empty((B, T, D), np.float32)
    align = np.empty((B, T, S), np.float32)
    for c in range(N_CORES):
        b, th = divmod(c, 2)
        attn[b, th * TH : (th + 1) * TH] = res.results[c]["attn"]
        align[b, th * TH : (th + 1) * TH] = res.results[c]["align"]
    return attn, align


# revision 48
# speedup vs baseline: 1.3098x; 1.0029x over previous
"""Bahdanau (additive) attention kernel for Trainium2, 8 NeuronCores.

Problem shapes: inp (B=4, T=128, D=512), context (B=4, S=512, D=512).
  wq   = inp @ Wq.T + bq                      (B,T,D)
  uh   = context @ Wc.T                       (B,S,D)
  align= einsum('btsd,d->bts', tanh(wq[:,:,None,:]+uh[:,None,:,:]), v)
  a    = softmax(align, -1)                   (B,T,S)
  c    = einsum('bts,bsd->btd', a, context)
  attn = concat([c, inp], -1) @ Wout.T + bout (B,T,D)
Returns (attn, a).

Sharding: 8 cores, core c handles batch b=c//2 and target-half th=c%2
(64 target positions per core). Weights replicated.

The dominant cost is the elementwise tanh over 16.8M elements per core.
This build splits that work across THREE engines per (t, d-chunk) unit
of [128 x 512]:
  - ACT: exact tanh on pre-summed tiles (adds from Pool or DVE)
  - Pool (gpsimd): tensor_scalar pre-adds for part of the ACT share
  - DVE: a clamped degree-7 odd-polynomial tanh evaluated by ONE custom
    DVE uop program (registered at import time): units get
    tensor_scalar (add bias, min B) -> tensor_scalar (max -B) ->
    custom op  g = (((c7 t + c5) t + c3) t + c1) * xc,  t = xc^2,
    which writes tanh-valued fp16 directly (max abs err ~8e-3 on the
    DVE share only; error budget checked end-to-end).
The per-t v-weighted d-reduction stays on the PE via one-hot shifted-Z
matmuls; softmax and the output projection are unchanged from the
ACT-only build.
"""

import numpy as np

import concourse.bacc as bacc
import concourse.tile as tile
from concourse import mybir
from concourse.bass import ds, ts
from concourse.bass_utils import run_bass_kernel_spmd
from concourse.masks import make_identity

F32 = mybir.dt.float32
F16 = mybir.dt.float16

B, T, S, D = 4, 128, 512, 512
N_CORES = 8
TH = T // 2  # 64 target positions per core
NCH = D // 128  # 4 partition chunks of the model dim
TBLK = 4  # target positions per main-loop block
NBLK = TH // TBLK

# ---- custom DVE op registration (degree-7 / degree-13 tanh) ----------------
from concourse import dve_ops as _dvo
from concourse.dve_spec import (
    Spec, Src0, Src1, C0, C1, C2, Latch, _has_src1, lower,
)
from concourse.dve_uop import DveOpSpec

# deg-7 odd minimax of tanh on [-B7, B7] with clamped tails
C7_, C5_, C3_, C1_ = (-0.002195027395932815, 0.0360726378920793,
                      -0.23082162587592386, 0.9654708772342129)
B7 = 2.40
# deg-13 (op pair) coefficients
P13, Q13, R13, W13 = (-41.46353605682129, 704.1415577602208,
                      -6400.397209947969, 34799.70434428201)
S13, U13, A13 = -126040.35564664593, 418583.06611382164, 2.3717188914682717e-06
B13 = 3.30


def _register(name, spec, subdim=False):
    for o in _dvo.OPS:
        if o.name == name:
            return o
    row = _dvo._CUSTOM_DVE_ROW_BASE + len(_dvo.OPS)
    assert row < 0x20
    _dvo._SUB_OPCODE_FOR_NAME[name] = row
    uops = lower(spec, ver="v3")
    sha = DveOpSpec(name=name, opcode=row, uops=uops,
                    rd1_en=_has_src1(spec)).sha("v3")
    op = _dvo.DveOp(name, spec, subdim=subdim, uops_sha={"v3": sha})
    _dvo.OPS.append(op)
    _dvo.CUSTOM_DVE_SPECS[name] = spec
    return op


def _tanh7_ref(in0, in1, s0, s1, imm2):
    xc = in0.astype(np.float32)
    t = xc * xc
    return ((s0 * t + s1) * t + imm2) * t * xc + in1 * xc


def _p13a_ref(in0, in1, s0, s1, imm2):
    t = in0.astype(np.float32) ** 2
    return (((t + s0) * t + s1) * t + imm2) * t + in1


def _p13b_ref(in0, in1, s0, s1, imm2):
    x1 = in1.astype(np.float32)
    t = x1 * x1
    return ((in0 * t + s0) * t + s1) * imm2 * x1


_t = Src0 * Src0
TANH7 = _register(
    "TANH7_ANT",
    Spec(body=(((C0 * _t + C1) * _t + C2) * _t + Latch(Src1)) * Src0,
         reference=_tanh7_ref),
)
TANH13A = _register(
    "TANH13A_ANT",
    Spec(body=(((_t + C0) * _t + C1) * _t + C2) * _t + Latch(Src1),
         reference=_p13a_ref),
)
_t2 = Src1 * Src1
TANH13B = _register(
    "TANH13B_ANT",
    Spec(body=((Src0 * _t2 + C0) * _t2 + C1) * C2 * Src1,
         reference=_p13b_ref),
)

# ---- per-block unit schedule ----------------------------------------------
# unit u = k*TBLK + tl (k-major). Tail units of each block go to the DVE
# polynomial stream; the rest are exact-ACT with pre-adds split Pool/DVE.
# The last block of each half is all-ACT so the softmax isn't gated on the
# (serial) DVE op chain; its DVE units are redistributed to mid blocks.
ND_TB = [4, 5, 5, 5, 5, 4, 5, 0,
         5, 5, 4, 5, 5, 4, 5, 0]   # DVE deg-7 units per block (tail units)
ND_MAX = max(ND_TB)
N_D13 = 0      # deg-13 units per block (before the deg-7 ones; 0 = disabled)
N_POOL = 6     # leading ACT units whose pre-add runs on Pool

_NC_CACHE = {}


def _build_nc():
    nc = bacc.Bacc("TRN2", target_bir_lowering=False, debug=False, num_devices=N_CORES)

    inpT = nc.dram_tensor("inpT", [D, TH], F16, kind="ExternalInput")
    ctxT = nc.dram_tensor("ctxT", [D, S], F16, kind="ExternalInput")
    wqT = nc.dram_tensor("wqT", [D, D], F16, kind="ExternalInput")
    wcT = nc.dram_tensor("wcT", [D, D], F16, kind="ExternalInput")
    woutT = nc.dram_tensor("woutT", [2 * D, D], F16, kind="ExternalInput")
    bq = nc.dram_tensor("bq", [D], F32, kind="ExternalInput")
    v = nc.dram_tensor("v", [D], F32, kind="ExternalInput")
    bout = nc.dram_tensor("bout", [D], F32, kind="ExternalInput")
    attn = nc.dram_tensor("attn", [TH, D], F32, kind="ExternalOutput")
    align = nc.dram_tensor("align", [TH, S], F32, kind="ExternalOutput")

    with tile.TileContext(nc) as tc:
        _emit(nc, tc, inpT, ctxT, wqT, wcT, woutT, bq, v, bout, attn, align)
    nc.compile()
    return nc


def _emit(nc, tc, inpT, ctxT, wqT, wcT, woutT, bq, v, bout, attn, align):
    Tanh = mybir.ActivationFunctionType.Tanh
    Exp = mybir.ActivationFunctionType.Exp
    Add = mybir.AluOpType.add
    Min = mybir.AluOpType.min
    Max = mybir.AluOpType.max
    NPOOL = N_POOL
    with (
        tc.tile_pool(name="persist", bufs=1) as P,
        tc.tile_pool(name="sums", bufs=3) as sums,
        tc.tile_pool(name="tanhs", bufs=3) as tanhs,
        tc.tile_pool(name="dvet", bufs=3) as dvet,
        tc.tile_pool(name="al_ps", bufs=1, space="PSUM") as al_ps,
        tc.tile_pool(name="mm_ps", bufs=2, space="PSUM") as mm_ps,
        tc.tile_pool(name="tr_ps", bufs=2, space="PSUM") as tr_ps,
        tc.tile_pool(name="o_ps", bufs=1, space="PSUM") as o_ps,
    ):
        # ---- persistent SBUF tiles + loads -------------------------------
        def load_wide(name, dram, engine=None):
            rows, F = dram.shape
            C = rows // 128
            t = P.tile([128, C * F], F16, name=name, tag=name)
            eng = engine or nc.sync
            eng.dma_start(
                out=t.rearrange("p (c f) -> p c f", c=C),
                in_=dram.ap().rearrange("(c p) f -> p c f", p=128),
            )
            return t

        ctxT_all = load_wide("ctxT_all", ctxT)
        wcT_all = P.tile([128, NCH * D], F16, name="wcT_all", tag="wcT_all")
        wcT_in3 = wcT.ap().rearrange("(c p) f -> p c f", p=128)
        wcT_out3 = wcT_all.rearrange("p (c f) -> p c f", c=NCH)
        wqT_all = P.tile([128, NCH * D], F16, name="wqT_all", tag="wqT_all")
        wqT_in3 = wqT.ap().rearrange("(c p) f -> p c f", p=128)
        wqT_out3 = wqT_all.rearrange("p (c f) -> p c f", c=NCH)
        # DMA order: k=0's weights (wcT cols 0:128 via h1, wqT cols 0:128)
        # first so prologue phase 0 starts ASAP; then the rest of wcT so
        # later uh chunks aren't stuck behind the full wqT transfer.
        nc.scalar.dma_start(out=wcT_out3[:, :, 0:256], in_=wcT_in3[:, :, 0:256])
        nc.scalar.dma_start(out=wqT_out3[:, :, 0:128], in_=wqT_in3[:, :, 0:128])
        inpT_all = load_wide("inpT_all", inpT)
        bq_sb = P.tile([128, NCH], F32, name="bq_sb", tag="bq_sb")
        nc.sync.dma_start(out=bq_sb, in_=bq.ap().rearrange("(k p) -> p k", p=128))
        v_sb = P.tile([128, NCH], F32, name="v_sb", tag="v_sb")
        nc.sync.dma_start(out=v_sb, in_=v.ap().rearrange("(k p) -> p k", p=128))
        nc.scalar.dma_start(out=wcT_out3[:, :, 256:512], in_=wcT_in3[:, :, 256:512])
        nc.scalar.dma_start(out=wqT_out3[:, :, 128:512], in_=wqT_in3[:, :, 128:512])
        ctxT_sb = [ctxT_all[:, ds(S * i, S)] for i in range(NCH)]
        wcT_sb = [wcT_all[:, ds(D * i, D)] for i in range(NCH)]
        wqT_sb = [wqT_all[:, ds(D * i, D)] for i in range(NCH)]
        inpT_sb = [inpT_all[:, ds(TH * i, TH)] for i in range(NCH)]

        # PE warmup: ramps the continuous-busy clock before real matmuls.
        warm_sb = P.tile([128, S], F16, name="warm_sb", tag="warm_sb")
        nc.vector.memset(warm_sb, 0.0)
        warm_ps = mm_ps.tile([128, S], F32, name="warm_ps", tag="uh_ps")
        for r in range(8):
            nc.tensor.matmul(warm_ps[0:64, :], lhsT=warm_sb[:, 0:64], rhs=warm_sb,
                             start=(r == 0), stop=(r == 7))

        # keep-warm: dep-free dummy matmuls fill PE idle gaps so the
        # continuous-busy clock stays at max rate (a cold restart doubles
        # every matmul's cycle time for 3us). They sit in the PE FIFO at
        # points where the PE would otherwise wait on a producer.
        hot_ps = o_ps.tile([64, 384], F32, name="hot_ps", tag="hot_ps", bufs=1)

        def keep_warm(n):
            for _ in range(n):
                nc.tensor.matmul(hot_ps, lhsT=warm_sb[:, 0:64],
                                 rhs=warm_sb[:, 0:384], start=True, stop=True)

        # constants for the DVE polynomial stream
        c1v = P.tile([128, 1], F32, name="c1v", tag="c1v")
        nc.gpsimd.memset(c1v, float(C1_))
        w13v = P.tile([128, 1], F32, name="w13v", tag="w13v")
        nc.gpsimd.memset(w13v, float(W13))

        # Z[k]: zeros with v chunk k at column 63 (shifted-window one-hot)
        Z = []
        for k in range(NCH):
            z = P.tile([128, 2 * TH - 1], F16, name=f"Z{k}", tag=f"Z{k}")
            nc.vector.memset(z, 0.0)
            Z.append(z)

        ident = P.tile([128, 128], F16, name="ident", tag="ident")
        make_identity(nc, ident)
        ones_sb = P.tile([1, TH], F16, name="ones_sb", tag="ones_sb")
        nc.vector.memset(ones_sb, 1.0)

        def load_epilogue_tensors():
            woutT_all = load_wide("woutT_all", woutT, nc.scalar)
            woutT_sb = [woutT_all[:, ds(D * i, D)] for i in range(2 * NCH)]
            bout_f32 = P.tile([1, D], F32, name="bout_f32", tag="bout_f32")
            nc.sync.dma_start(
                out=bout_f32, in_=bout.ap().rearrange("(o f) -> o f", o=1)
            )
            bout_sb = P.tile([1, D], F16, name="bout_sb", tag="bout_sb")
            nc.vector.tensor_copy(bout_sb, bout_f32)
            return woutT_sb, bout_sb

        # ---- uh^T[e,s] = Wc @ context^T and wqb^T[e,t] = Wq @ inp^T + bq -
        uh_sb = [None] * NCH
        wqb_sb = [None] * NCH

        def prologue_phase(ks):
            for k in ks:
                ps = mm_ps.tile([128, S], F32, name="uh_ps", tag="uh_ps")
                for j in range(NCH):
                    nc.tensor.matmul(
                        ps,
                        lhsT=wcT_sb[j][:, ts(k, 128)],
                        rhs=ctxT_sb[j],
                        start=(j == 0),
                        stop=(j == NCH - 1),
                    )
                wps = tr_ps.tile([128, TH], F32, name="wq_ps", tag="wq_ps", bufs=1)
                for j in range(NCH):
                    nc.tensor.matmul(
                        wps,
                        lhsT=wqT_sb[j][:, ts(k, 128)],
                        rhs=inpT_sb[j],
                        start=(j == 0),
                        stop=(j == NCH - 1),
                    )
                u = P.tile([128, S], F16, name=f"uh{k}", tag=f"uh{k}")
                nc.vector.tensor_copy(u, ps)
                uh_sb[k] = u
                w = P.tile([128, TH], F32, name=f"wqb{k}", tag=f"wqb{k}")
                nc.vector.tensor_scalar_add(w, wps, bq_sb[:, k : k + 1])
                wqb_sb[k] = w

        prologue_phase([0])

        v16 = P.tile([128, NCH], F16, name="v16", tag="v16")
        nc.vector.tensor_copy(v16, v_sb)
        for k in range(NCH):
            nc.vector.tensor_copy(Z[k][:, TH - 1 : TH], v16[:, k : k + 1])

        # ---- main loop ---------------------------------------------------
        HT = TH // 2  # 32 rows per align half
        al_half = [
            al_ps.tile([HT, S], F32, name=f"al{h}", tag=f"al{h}") for h in range(2)
        ]
        woutT_sb = bout_sb = None
        blkmm = {}  # tb -> emitted Z-matmuls (16 per block closes the group)

        a16_h = {}
        alT_ps_h = {}

        def epilogue_softmax_g(src, h2, r0, nr, kw):
            # softmax + transposes for rows [r0, r0+nr) of half h2, sourced
            # from align tile `src`. All intermediates are base-0 per-group
            # tiles (matmul/transpose operands need base partition 0/32/64).
            gid = f"{h2}_{r0}"
            rows = ds(h2 * HT + r0, nr)
            p_h = P.tile([nr, S], F32, name=f"p{gid}", tag=f"p{gid}")
            ssum = P.tile([nr, 1], F32, name=f"ssum{gid}", tag=f"ssum{gid}")
            nc.scalar.activation(p_h, src[0:nr, :], Exp,
                                 accum_out=ssum[:, 0:1])
            rcp = P.tile([nr, 1], F32, name=f"rcp{gid}", tag=f"rcp{gid}")
            nc.vector.reciprocal(rcp, ssum)
            # align first so its (long-latency) DMA overlaps the attn tail
            nc.vector.tensor_scalar_mul(align_sb[rows, :], p_h, rcp[:, 0:1])
            nc.sync.dma_start(out=align.ap()[rows, :], in_=align_sb[rows, :])
            a16 = P.tile([nr, S], F16, name=f"a16_{gid}", tag=f"a16_{gid}")
            nc.vector.tensor_scalar_mul(a16, p_h, rcp[:, 0:1])
            if h2 not in alT_ps_h:
                alT_ps_h[h2] = tr_ps.tile(
                    [128, NCH * HT], F16, name="alT_ps", tag="alT_ps", bufs=1
                )
            keep_warm(kw)
            for i in range(NCH):
                nc.tensor.transpose(
                    alT_ps_h[h2][:, ds(i * HT + r0, nr)],
                    a16[:, ts(i, 128)], ident[0:nr, 0:nr]
                )

        def epilogue_close(h2):
            alT = P.tile([128, NCH * HT], F16, name=f"alT{h2}", tag=f"alT{h2}")
            nc.vector.tensor_copy(alT, alT_ps_h[h2])
            out_ps = out_ps_h[h2]
            for sc in range(NCH):
                nc.tensor.matmul(
                    out_ps,
                    lhsT=alT[:, ts(sc, HT)],
                    rhs=M_sb[sc],
                    start=False,
                    stop=(sc == NCH - 1),
                )

        def epilogue_attn(h2):
            # copies split ACT/DVE (both can read PSUM); DMAs on the scalar
            # queue so they don't serialize behind the align DMA on SP
            rows = ds(h2 * HT, HT)
            out_ps = out_ps_h[h2]
            for eh in range(2):
                ecols = ds(eh * (D // 2), D // 2)
                if eh == 0:
                    nc.scalar.copy(attn_sb[rows, ecols], out_ps[:, ecols])
                else:
                    nc.vector.tensor_copy(attn_sb[rows, ecols], out_ps[:, ecols])
                nc.scalar.dma_start(
                    out=attn.ap()[rows, ecols], in_=attn_sb[rows, ecols]
                )

        out_ps_h = {}
        M_sb = [None] * NCH

        def emit_M_chunk(sc, woutT_sb):
            ps = mm_ps.tile([128, S], F32, name="M_ps", tag="uh_ps")
            for j in range(NCH):
                nc.tensor.matmul(
                    ps,
                    lhsT=ctxT_all[:, ds(S * j + 128 * sc, 128)],
                    rhs=woutT_sb[j],
                    start=(j == 0),
                    stop=(j == NCH - 1),
                )
            m = P.tile([128, S], F16, name=f"M{sc}", tag=f"M{sc}")
            nc.vector.tensor_copy(m, ps)
            M_sb[sc] = m

        def out_early(h2, woutT_sb, bout_sb):
            rows = ds(h2 * HT, HT)
            out_ps = o_ps.tile([HT, D], F32, name="out_ps", tag="out_ps", bufs=1)
            nc.tensor.matmul(
                out_ps, lhsT=ones_sb[:, 0:HT], rhs=bout_sb, start=True, stop=False
            )
            for f in range(NCH, 2 * NCH):
                nc.tensor.matmul(
                    out_ps,
                    lhsT=inpT_sb[f - NCH][:, rows],
                    rhs=woutT_sb[f],
                    start=False,
                    stop=False,
                )
            out_ps_h[h2] = out_ps

        def zmm(tb, k, tl, rhs_slice):
            # one Z-matmul accumulating row (tb%HB)*4+tl of al_half[tb//HB]
            h2 = tb // HB
            t_loc = (tb % HB) * TBLK + tl
            n = blkmm.get(h2, 0)
            nc.tensor.matmul(
                al_half[h2],
                lhsT=Z[k][:, TH - 1 - t_loc : TH - 1 - t_loc + HT],
                rhs=rhs_slice,
                start=(n == 0),
                stop=(n == 8 * 16 - 1),
            )
            blkmm[h2] = n + 1

        align_sb = P.tile([TH, S], F32, name="align_sb", tag="align_sb")
        attn_sb = P.tile([TH, D], F32, name="attn_sb", tag="attn_sb")
        HB = NBLK // 2  # main-loop blocks per align half

        blk = {}  # tb -> dict of tiles

        def unit_add(tb, u, ui, eng, Bclamp=None):
            # pre-add for unit u; ACT units (ui = index into sum_t),
            # DVE units (ui = index into xm_t, with min clamp)
            b = blk[tb]
            k, tl = divmod(u, TBLK)
            t = tb * TBLK + tl
            if Bclamp is None:
                eng.tensor_scalar(
                    out=b["sum"][:, ds(ui * S, S)], in0=uh_sb[k],
                    scalar1=wqb_sb[k][:, t : t + 1], scalar2=None, op0=Add,
                )
            else:
                eng.tensor_scalar(
                    out=b["xm"][:, ds(ui * S, S)], in0=uh_sb[k],
                    scalar1=wqb_sb[k][:, t : t + 1], scalar2=Bclamp,
                    op0=Add, op1=Min,
                )

        def stage1(tb):
            # producers: tile allocation + all pre-adds / TS1s for block tb
            ND = ND_TB[tb]
            NA = 16 - ND
            b = blk[tb] = dict(
                sum=sums.tile([128, 16 * S], F16, name="sum_t", tag="sum_t"),
                tanh=tanhs.tile([128, 16 * S], F16, name="tanh_t", tag="tanh_t"),
            )
            if ND:
                b["xm"] = dvet.tile([128, ND_MAX * S], F16, name="xm_t", tag="xm_t")
                b["xc"] = dvet.tile([128, ND_MAX * S], F16, name="xc_t", tag="xc_t")
                b["g"] = dvet.tile([128, ND_MAX * S], F16, name="g_t", tag="g_t")
            if N_D13:
                b["p4"] = dvet.tile([128, N_D13 * S], F32, name="p4_t", tag="p4_t")
            for u in range(NPOOL):
                unit_add(tb, u, u, nc.gpsimd)
            for u in range(NPOOL, NA):
                unit_add(tb, u, u, nc.vector)
            for i, u in enumerate(range(NA, 16)):
                unit_add(tb, u, i, nc.vector, Bclamp=B13 if i < N_D13 else B7)

        def stage2(tb):
            # TS2 + custom ops over the packed DVE units of block tb
            ND = ND_TB[tb]
            N_D7 = ND - N_D13
            if not ND:
                return
            b = blk[tb]
            xm_t, xc_t, g_t = b["xm"], b["xc"], b["g"]
            if N_D13:
                sl = ds(0, N_D13 * S)
                nc.vector.tensor_scalar(
                    out=xc_t[:, sl], in0=xm_t[:, sl],
                    scalar1=float(-B13), scalar2=None, op0=Max,
                )
                nc.vector._custom_dve(
                    TANH13A, out=b["p4"], in0=xc_t[:, sl], in1=w13v,
                    s0=float(P13), s1=float(Q13), imm2=float(R13),
                )
                nc.vector._custom_dve(
                    TANH13B, out=g_t[:, sl], in0=b["p4"], in1=xc_t[:, sl],
                    s0=float(S13), s1=float(U13), imm2=float(A13),
                )
            if N_D7:
                sl = ds(N_D13 * S, N_D7 * S)
                nc.vector.tensor_scalar(
                    out=xc_t[:, sl], in0=xm_t[:, sl],
                    scalar1=float(-B7), scalar2=None, op0=Max,
                )
                nc.vector._custom_dve(
                    TANH7, out=g_t[:, sl], in0=xc_t[:, sl], in1=c1v,
                    s0=float(C7_), s1=float(C5_), imm2=float(C3_),
                )

        def act_stage(tb):
            # one tanh instruction per block: all producers ran >=1 block
            # ago (software pipelining), so the coarse dep costs nothing
            # and the per-instruction overhead is paid once.
            NA = 16 - ND_TB[tb]
            b = blk[tb]
            nc.scalar.activation(
                b["tanh"][:, ds(0, NA * S)], b["sum"][:, ds(0, NA * S)], Tanh
            )

        def pe_units(tb, units, tile_, base):
            for i, u in enumerate(units):
                k, tl = divmod(u, TBLK)
                zmm(tb, k, tl, tile_[:, ds((base + i) * S, S)])

        def pe_stage(tb, kw=9):
            ND = ND_TB[tb]
            NA = 16 - ND
            b = blk[tb]
            pe_units(tb, list(range(0, NA)), b["tanh"], 0)
            if ND:
                pe_units(tb, list(range(NA, 16)), b["g"], 0)
            keep_warm(kw)
            del blk[tb]

        def emit_block0():
            # chunk-at-a-time with just-in-time prologue phases (all-ACT)
            ND = ND_TB[0]
            NA = 16 - ND
            b = blk[0] = dict(
                sum=sums.tile([128, 16 * S], F16, name="sum_t", tag="sum_t"),
                tanh=tanhs.tile([128, 16 * S], F16, name="tanh_t", tag="tanh_t"),
            )
            if ND:
                b["xm"] = dvet.tile([128, ND_MAX * S], F16, name="xm_t", tag="xm_t")
                b["xc"] = dvet.tile([128, ND_MAX * S], F16, name="xc_t", tag="xc_t")
                b["g"] = dvet.tile([128, ND_MAX * S], F16, name="g_t", tag="g_t")
            if N_D13:
                b["p4"] = dvet.tile([128, N_D13 * S], F32, name="p4_t", tag="p4_t")
            # all adds + remaining prologue phases first: the uh_k matmuls
            # must not sit behind Z-matmuls (which wait on ACT) in the PE
            # FIFO. Block-0 adds are all-DVE: Pool's first work is block 1,
            # so the scheduler can't starve block-0's tanh stream with it.
            for k in range(NCH):
                for tl in range(TBLK):
                    u = k * TBLK + tl
                    if u < NA:
                        unit_add(0, u, u, nc.vector)
                    else:
                        unit_add(0, u, u - NA, nc.vector,
                                 Bclamp=B13 if u - NA < N_D13 else B7)
                if k + 1 < NCH:
                    prologue_phase([k + 1])
            for k in range(NCH):
                ka = [u for u in range(k * TBLK, (k + 1) * TBLK) if u < NA]
                if ka:
                    lo, hi = ka[0], ka[-1] + 1
                    nc.scalar.activation(
                        b["tanh"][:, ds(lo * S, (hi - lo) * S)],
                        b["sum"][:, ds(lo * S, (hi - lo) * S)], Tanh,
                    )
                    pe_units(0, ka, b["tanh"], lo)
            keep_warm(10)

        def finish_block0():
            ND = ND_TB[0]
            NA = 16 - ND
            if ND:
                stage2(0)
                pe_units(0, list(range(NA, 16)), blk[0]["g"], 0)
            del blk[0]

        # software-pipelined main loop: block n's producers (stage1) are
        # emitted ~2 blocks ahead of its DVE op chain (stage2), so the ACT
        # stream is never queued behind the custom-op work on the DVE FIFO.
        emit_block0()
        woutT_sb, bout_sb = load_epilogue_tensors()
        stage1(1)
        finish_block0()

        def last_block(tb):
            # final block of the run: all-ACT; emit tanh+matmuls in 4-unit
            # quarters so the align accumulation finishes (and the epilogue
            # starts) right after the last quarter instead of after one
            # monolithic 16-unit activation.
            b = blk[tb]
            for q in range(4):
                qs = ds(q * 4 * S, 4 * S)
                nc.scalar.activation(b["tanh"][:, qs], b["sum"][:, qs], Tanh)
                pe_units(tb, list(range(q * 4, q * 4 + 4)), b["tanh"], q * 4)
            keep_warm(4)
            del blk[tb]

        for tb in range(1, NBLK):
            if tb == NBLK - 1:
                stage2(tb)
                last_block(tb)
                epilogue_softmax_g(al_half[1], 1, 0, HT, kw=5)
                epilogue_close(1)
                epilogue_attn(1)
                break
            stage2(tb)
            act_stage(tb)
            pe_stage(tb)
            if tb + 1 < NBLK and (tb + 1) not in blk:
                stage1(tb + 1)
            if tb + 2 < NBLK:
                stage1(tb + 2)
            if 2 <= tb <= 5:
                emit_M_chunk(tb - 2, woutT_sb)
            if tb == HB - 4:
                out_early(0, woutT_sb, bout_sb)
            if tb == NBLK - 4:
                out_early(1, woutT_sb, bout_sb)
            # half-0 epilogue is emitted one block late so its cross-engine
            # chain doesn't head-of-line-block the next block's producers
            if tb == HB:
                epilogue_softmax_g(al_half[0], 0, 0, HT, kw=4)
                epilogue_close(0)
            if tb == HB + 1:
                epilogue_attn(0)
            if tb == NBLK - 1:
                epilogue_softmax(1, kw=7)
                epilogue_attn(1)


def get_nc():
    if "nc" not in _NC_CACHE:
        _NC_CACHE["nc"] = _build_nc()
    return _NC_CACHE["nc"]


def make_in_maps(inp, context, Wq, bq, Wc, v, Wout, bout):
    inp = np.asarray(inp, np.float32)
    context = np.asarray(context, np.float32)
    Wq = np.asarray(Wq, np.float32)
    bq = np.asarray(bq, np.float32)
    Wc = np.asarray(Wc, np.float32)
    v = np.asarray(v, np.float32)
    Wout = np.asarray(Wout, np.float32)
    bout = np.asarray(bout, np.float32)

    wqT = np.ascontiguousarray(Wq.T).astype(np.float16)
    wcT = np.ascontiguousarray(Wc.T).astype(np.float16)
    woutT = np.ascontiguousarray(Wout.T).astype(np.float16)
    in_maps = []
    for c in range(N_CORES):
        b, th = divmod(c, 2)
        in_maps.append(
            {
                "inpT": np.ascontiguousarray(
                    inp[b, th * TH : (th + 1) * TH].T
                ).astype(np.float16),
                "ctxT": np.ascontiguousarray(context[b].T).astype(np.float16),
                "wqT": wqT,
                "wcT": wcT,
                "woutT": woutT,
                "bq": bq,
                "v": v,
                "bout": bout,
            }
        )
    return in_maps


def run_on_device(in_maps, **kwargs):
    nc = get_nc()
    return run_bass_kernel_spmd(nc, in_maps, core_ids=list(range(N_CORES)), **kwargs)


def kernel(inp, context, Wq, bq, Wc, v, Wout, bout):
    in_maps = make_in_maps(inp, context, Wq, bq, Wc, v, Wout, bout)
    res = run_on_device(in_maps)
    attn = np.empty((B, T, D), np.float32)
    align = np.empty((B, T, S), np.float32)
    for c in range(N_CORES):
        b, th = divmod(c, 2)
        attn[b, th * TH : (th + 1) * TH] = res.results[c]["attn"]
        align[b, th * TH : (th + 1) * TH] = res.results[c]["align"]
    return attn, align


# revision 54
# speedup vs baseline: 1.3126x; 1.0021x over previous
"""Bahdanau (additive) attention kernel for Trainium2, 8 NeuronCores.

Problem shapes: inp (B=4, T=128, D=512), context (B=4, S=512, D=512).
  wq   = inp @ Wq.T + bq                      (B,T,D)
  uh   = context @ Wc.T                       (B,S,D)
  align= einsum('btsd,d->bts', tanh(wq[:,:,None,:]+uh[:,None,:,:]), v)
  a    = softmax(align, -1)                   (B,T,S)
  c    = einsum('bts,bsd->btd', a, context)
  attn = concat([c, inp], -1) @ Wout.T + bout (B,T,D)
Returns (attn, a).

Sharding: 8 cores, core c handles batch b=c//2 and target-half th=c%2
(64 target positions per core). Weights replicated.

The dominant cost is the elementwise tanh over 16.8M elements per core.
This build splits that work across THREE engines per (t, d-chunk) unit
of [128 x 512]:
  - ACT: exact tanh on pre-summed tiles (adds from Pool or DVE)
  - Pool (gpsimd): tensor_scalar pre-adds for part of the ACT share
  - DVE: a clamped degree-7 odd-polynomial tanh evaluated by ONE custom
    DVE uop program (registered at import time): units get
    tensor_scalar (add bias, min B) -> tensor_scalar (max -B) ->
    custom op  g = (((c7 t + c5) t + c3) t + c1) * xc,  t = xc^2,
    which writes tanh-valued fp16 directly (max abs err ~8e-3 on the
    DVE share only; error budget checked end-to-end).
The per-t v-weighted d-reduction stays on the PE via one-hot shifted-Z
matmuls; softmax and the output projection are unchanged from the
ACT-only build.
"""

import numpy as np

import concourse.bacc as bacc
import concourse.tile as tile
from concourse import mybir
from concourse.bass import ds, ts
from concourse.bass_utils import run_bass_kernel_spmd
from concourse.masks import make_identity

F32 = mybir.dt.float32
F16 = mybir.dt.float16

B, T, S, D = 4, 128, 512, 512
N_CORES = 8
TH = T // 2  # 64 target positions per core
NCH = D // 128  # 4 partition chunks of the model dim
TBLK = 4  # target positions per main-loop block
NBLK = TH // TBLK

# ---- custom DVE op registration (degree-7 / degree-13 tanh) ----------------
from concourse import dve_ops as _dvo
from concourse.dve_spec import (
    Spec, Src0, Src1, C0, C1, C2, Latch, _has_src1, lower,
)
from concourse.dve_uop import DveOpSpec

# deg-7 odd minimax of tanh on [-B7, B7] with clamped tails
C7_, C5_, C3_, C1_ = (-0.002195027395932815, 0.0360726378920793,
                      -0.23082162587592386, 0.9654708772342129)
B7 = 2.40
# deg-13 (op pair) coefficients
P13, Q13, R13, W13 = (-41.46353605682129, 704.1415577602208,
                      -6400.397209947969, 34799.70434428201)
S13, U13, A13 = -126040.35564664593, 418583.06611382164, 2.3717188914682717e-06
B13 = 3.30


def _register(name, spec, subdim=False):
    for o in _dvo.OPS:
        if o.name == name:
            return o
    row = _dvo._CUSTOM_DVE_ROW_BASE + len(_dvo.OPS)
    assert row < 0x20
    _dvo._SUB_OPCODE_FOR_NAME[name] = row
    uops = lower(spec, ver="v3")
    sha = DveOpSpec(name=name, opcode=row, uops=uops,
                    rd1_en=_has_src1(spec)).sha("v3")
    op = _dvo.DveOp(name, spec, subdim=subdim, uops_sha={"v3": sha})
    _dvo.OPS.append(op)
    _dvo.CUSTOM_DVE_SPECS[name] = spec
    return op


def _tanh7_ref(in0, in1, s0, s1, imm2):
    xc = in0.astype(np.float32)
    t = xc * xc
    return ((s0 * t + s1) * t + imm2) * t * xc + in1 * xc


def _p13a_ref(in0, in1, s0, s1, imm2):
    t = in0.astype(np.float32) ** 2
    return (((t + s0) * t + s1) * t + imm2) * t + in1


def _p13b_ref(in0, in1, s0, s1, imm2):
    x1 = in1.astype(np.float32)
    t = x1 * x1
    return ((in0 * t + s0) * t + s1) * imm2 * x1


_t = Src0 * Src0
TANH7 = _register(
    "TANH7_ANT",
    Spec(body=(((C0 * _t + C1) * _t + C2) * _t + Latch(Src1)) * Src0,
         reference=_tanh7_ref),
)
TANH13A = _register(
    "TANH13A_ANT",
    Spec(body=(((_t + C0) * _t + C1) * _t + C2) * _t + Latch(Src1),
         reference=_p13a_ref),
)
_t2 = Src1 * Src1
TANH13B = _register(
    "TANH13B_ANT",
    Spec(body=((Src0 * _t2 + C0) * _t2 + C1) * C2 * Src1,
         reference=_p13b_ref),
)

# ---- per-block unit schedule ----------------------------------------------
# unit u = k*TBLK + tl (k-major). Tail units of each block go to the DVE
# polynomial stream; the rest are exact-ACT with pre-adds split Pool/DVE.
# The last block of each half is all-ACT so the softmax isn't gated on the
# (serial) DVE op chain; its DVE units are redistributed to mid blocks.
ND_TB = [4, 4, 5, 5, 5, 5, 5, 0,
         5, 5, 4, 5, 5, 4, 5, 0]   # DVE deg-7 units per block (tail units)
ND_MAX = max(ND_TB)
N_D13 = 0      # deg-13 units per block (before the deg-7 ones; 0 = disabled)
N_POOL = 6     # leading ACT units whose pre-add runs on Pool

_NC_CACHE = {}


def _build_nc():
    nc = bacc.Bacc("TRN2", target_bir_lowering=False, debug=False, num_devices=N_CORES)

    inpT = nc.dram_tensor("inpT", [D, TH], F16, kind="ExternalInput")
    ctxT = nc.dram_tensor("ctxT", [D, S], F16, kind="ExternalInput")
    wqT = nc.dram_tensor("wqT", [D, D], F16, kind="ExternalInput")
    wcT = nc.dram_tensor("wcT", [D, D], F16, kind="ExternalInput")
    woutT = nc.dram_tensor("woutT", [2 * D, D], F16, kind="ExternalInput")
    bq = nc.dram_tensor("bq", [D], F32, kind="ExternalInput")
    v = nc.dram_tensor("v", [D], F32, kind="ExternalInput")
    bout = nc.dram_tensor("bout", [D], F32, kind="ExternalInput")
    attn = nc.dram_tensor("attn", [TH, D], F32, kind="ExternalOutput")
    align = nc.dram_tensor("align", [TH, S], F32, kind="ExternalOutput")

    with tile.TileContext(nc) as tc:
        _emit(nc, tc, inpT, ctxT, wqT, wcT, woutT, bq, v, bout, attn, align)
    nc.compile()
    return nc


def _emit(nc, tc, inpT, ctxT, wqT, wcT, woutT, bq, v, bout, attn, align):
    Tanh = mybir.ActivationFunctionType.Tanh
    Exp = mybir.ActivationFunctionType.Exp
    Add = mybir.AluOpType.add
    Min = mybir.AluOpType.min
    Max = mybir.AluOpType.max
    NPOOL = N_POOL
    with (
        tc.tile_pool(name="persist", bufs=1) as P,
        tc.tile_pool(name="sums", bufs=3) as sums,
        tc.tile_pool(name="tanhs", bufs=3) as tanhs,
        tc.tile_pool(name="dvet", bufs=3) as dvet,
        tc.tile_pool(name="al_ps", bufs=1, space="PSUM") as al_ps,
        tc.tile_pool(name="mm_ps", bufs=2, space="PSUM") as mm_ps,
        tc.tile_pool(name="tr_ps", bufs=2, space="PSUM") as tr_ps,
        tc.tile_pool(name="o_ps", bufs=1, space="PSUM") as o_ps,
    ):
        # ---- persistent SBUF tiles + loads -------------------------------
        def load_wide(name, dram, engine=None):
            rows, F = dram.shape
            C = rows // 128
            t = P.tile([128, C * F], F16, name=name, tag=name)
            eng = engine or nc.sync
            eng.dma_start(
                out=t.rearrange("p (c f) -> p c f", c=C),
                in_=dram.ap().rearrange("(c p) f -> p c f", p=128),
            )
            return t

        ctxT_all = load_wide("ctxT_all", ctxT)
        wcT_all = P.tile([128, NCH * D], F16, name="wcT_all", tag="wcT_all")
        wcT_in3 = wcT.ap().rearrange("(c p) f -> p c f", p=128)
        wcT_out3 = wcT_all.rearrange("p (c f) -> p c f", c=NCH)
        wqT_all = P.tile([128, NCH * D], F16, name="wqT_all", tag="wqT_all")
        wqT_in3 = wqT.ap().rearrange("(c p) f -> p c f", p=128)
        wqT_out3 = wqT_all.rearrange("p (c f) -> p c f", c=NCH)
        # DMA order: k=0's weights (wcT cols 0:128 via h1, wqT cols 0:128)
        # first so prologue phase 0 starts ASAP; then the rest of wcT so
        # later uh chunks aren't stuck behind the full wqT transfer.
        nc.scalar.dma_start(out=wcT_out3[:, :, 0:256], in_=wcT_in3[:, :, 0:256])
        nc.scalar.dma_start(out=wqT_out3[:, :, 0:128], in_=wqT_in3[:, :, 0:128])
        inpT_all = load_wide("inpT_all", inpT)
        bq_sb = P.tile([128, NCH], F32, name="bq_sb", tag="bq_sb")
        nc.sync.dma_start(out=bq_sb, in_=bq.ap().rearrange("(k p) -> p k", p=128))
        v_sb = P.tile([128, NCH], F32, name="v_sb", tag="v_sb")
        nc.sync.dma_start(out=v_sb, in_=v.ap().rearrange("(k p) -> p k", p=128))
        nc.scalar.dma_start(out=wcT_out3[:, :, 256:512], in_=wcT_in3[:, :, 256:512])
        nc.scalar.dma_start(out=wqT_out3[:, :, 128:512], in_=wqT_in3[:, :, 128:512])
        ctxT_sb = [ctxT_all[:, ds(S * i, S)] for i in range(NCH)]
        wcT_sb = [wcT_all[:, ds(D * i, D)] for i in range(NCH)]
        wqT_sb = [wqT_all[:, ds(D * i, D)] for i in range(NCH)]
        inpT_sb = [inpT_all[:, ds(TH * i, TH)] for i in range(NCH)]

        # PE warmup: ramps the continuous-busy clock before real matmuls.
        warm_sb = P.tile([128, S], F16, name="warm_sb", tag="warm_sb")
        nc.vector.memset(warm_sb, 0.0)
        warm_ps = mm_ps.tile([128, S], F32, name="warm_ps", tag="uh_ps")
        for r in range(8):
            nc.tensor.matmul(warm_ps[0:64, :], lhsT=warm_sb[:, 0:64], rhs=warm_sb,
                             start=(r == 0), stop=(r == 7))

        # keep-warm: dep-free dummy matmuls fill PE idle gaps so the
        # continuous-busy clock stays at max rate (a cold restart doubles
        # every matmul's cycle time for 3us). They sit in the PE FIFO at
        # points where the PE would otherwise wait on a producer.
        hot_ps = o_ps.tile([64, 384], F32, name="hot_ps", tag="hot_ps", bufs=1)

        def keep_warm(n):
            for _ in range(n):
                nc.tensor.matmul(hot_ps, lhsT=warm_sb[:, 0:64],
                                 rhs=warm_sb[:, 0:384], start=True, stop=True)

        # constants for the DVE polynomial stream
        c1v = P.tile([128, 1], F32, name="c1v", tag="c1v")
        nc.gpsimd.memset(c1v, float(C1_))
        w13v = P.tile([128, 1], F32, name="w13v", tag="w13v")
        nc.gpsimd.memset(w13v, float(W13))

        # Z[k]: zeros with v chunk k at column 63 (shifted-window one-hot)
        Z = []
        for k in range(NCH):
            z = P.tile([128, 2 * TH - 1], F16, name=f"Z{k}", tag=f"Z{k}")
            nc.vector.memset(z, 0.0)
            Z.append(z)

        ident = P.tile([128, 128], F16, name="ident", tag="ident")
        make_identity(nc, ident)
        ones_sb = P.tile([1, TH], F16, name="ones_sb", tag="ones_sb")
        nc.vector.memset(ones_sb, 1.0)

        def load_epilogue_tensors():
            woutT_all = load_wide("woutT_all", woutT, nc.scalar)
            woutT_sb = [woutT_all[:, ds(D * i, D)] for i in range(2 * NCH)]
            bout_f32 = P.tile([1, D], F32, name="bout_f32", tag="bout_f32")
            nc.sync.dma_start(
                out=bout_f32, in_=bout.ap().rearrange("(o f) -> o f", o=1)
            )
            bout_sb = P.tile([1, D], F16, name="bout_sb", tag="bout_sb")
            nc.vector.tensor_copy(bout_sb, bout_f32)
            return woutT_sb, bout_sb

        # ---- uh^T[e,s] = Wc @ context^T and wqb^T[e,t] = Wq @ inp^T + bq -
        uh_sb = [None] * NCH
        wqb_sb = [None] * NCH

        def prologue_phase(ks):
            for k in ks:
                ps = mm_ps.tile([128, S], F32, name="uh_ps", tag="uh_ps")
                for j in range(NCH):
                    nc.tensor.matmul(
                        ps,
                        lhsT=wcT_sb[j][:, ts(k, 128)],
                        rhs=ctxT_sb[j],
                        start=(j == 0),
                        stop=(j == NCH - 1),
                    )
                wps = tr_ps.tile([128, TH], F32, name="wq_ps", tag="wq_ps", bufs=1)
                for j in range(NCH):
                    nc.tensor.matmul(
                        wps,
                        lhsT=wqT_sb[j][:, ts(k, 128)],
                        rhs=inpT_sb[j],
                        start=(j == 0),
                        stop=(j == NCH - 1),
                    )
                u = P.tile([128, S], F16, name=f"uh{k}", tag=f"uh{k}")
                nc.vector.tensor_copy(u, ps)
                uh_sb[k] = u
                w = P.tile([128, TH], F32, name=f"wqb{k}", tag=f"wqb{k}")
                nc.vector.tensor_scalar_add(w, wps, bq_sb[:, k : k + 1])
                wqb_sb[k] = w

        prologue_phase([0])

        v16 = P.tile([128, NCH], F16, name="v16", tag="v16")
        nc.vector.tensor_copy(v16, v_sb)
        for k in range(NCH):
            nc.vector.tensor_copy(Z[k][:, TH - 1 : TH], v16[:, k : k + 1])

        # ---- main loop ---------------------------------------------------
        HT = TH // 2  # 32 rows per align half
        al_half = [
            al_ps.tile([HT, S], F32, name=f"al{h}", tag=f"al{h}") for h in range(2)
        ]
        woutT_sb = bout_sb = None
        blkmm = {}  # tb -> emitted Z-matmuls (16 per block closes the group)

        a16_h = {}
        alT_ps_h = {}

        def epilogue_softmax_g(src, h2, r0, nr, kw):
            # softmax + transposes for rows [r0, r0+nr) of half h2, sourced
            # from align tile `src`. All intermediates are base-0 per-group
            # tiles (matmul/transpose operands need base partition 0/32/64).
            gid = f"{h2}_{r0}"
            rows = ds(h2 * HT + r0, nr)
            p_h = P.tile([nr, S], F32, name=f"p{gid}", tag=f"p{gid}")
            ssum = P.tile([nr, 1], F32, name=f"ssum{gid}", tag=f"ssum{gid}")
            nc.scalar.activation(p_h, src[0:nr, :], Exp,
                                 accum_out=ssum[:, 0:1])
            rcp = P.tile([nr, 1], F32, name=f"rcp{gid}", tag=f"rcp{gid}")
            nc.vector.reciprocal(rcp, ssum)
            # align first so its (long-latency) DMA overlaps the attn tail
            nc.vector.tensor_scalar_mul(align_sb[rows, :], p_h, rcp[:, 0:1])
            nc.sync.dma_start(out=align.ap()[rows, :], in_=align_sb[rows, :])
            a16 = P.tile([nr, S], F16, name=f"a16_{gid}", tag=f"a16_{gid}")
            nc.vector.tensor_scalar_mul(a16, p_h, rcp[:, 0:1])
            if h2 not in alT_ps_h:
                alT_ps_h[h2] = tr_ps.tile(
                    [128, NCH * HT], F16, name="alT_ps", tag="alT_ps", bufs=1
                )
            keep_warm(kw)
            for i in range(NCH):
                nc.tensor.transpose(
                    alT_ps_h[h2][:, ds(i * HT + r0, nr)],
                    a16[:, ts(i, 128)], ident[0:nr, 0:nr]
                )

        def epilogue_close(h2):
            alT = P.tile([128, NCH * HT], F16, name=f"alT{h2}", tag=f"alT{h2}")
            nc.vector.tensor_copy(alT, alT_ps_h[h2])
            out_ps = out_ps_h[h2]
            for sc in range(NCH):
                nc.tensor.matmul(
                    out_ps,
                    lhsT=alT[:, ts(sc, HT)],
                    rhs=M_sb[sc],
                    start=False,
                    stop=(sc == NCH - 1),
                )

        def epilogue_attn(h2):
            # copies split ACT/DVE (both can read PSUM); DMAs on the scalar
            # queue so they don't serialize behind the align DMA on SP
            rows = ds(h2 * HT, HT)
            out_ps = out_ps_h[h2]
            for eh in range(2):
                ecols = ds(eh * (D // 2), D // 2)
                if eh == 0:
                    nc.scalar.copy(attn_sb[rows, ecols], out_ps[:, ecols])
                else:
                    nc.vector.tensor_copy(attn_sb[rows, ecols], out_ps[:, ecols])
                (nc.sync if eh else nc.scalar).dma_start(
                    out=attn.ap()[rows, ecols], in_=attn_sb[rows, ecols]
                )

        out_ps_h = {}
        M_sb = [None] * NCH

        def emit_M_chunk(sc, woutT_sb):
            ps = mm_ps.tile([128, S], F32, name="M_ps", tag="uh_ps")
            for j in range(NCH):
                nc.tensor.matmul(
                    ps,
                    lhsT=ctxT_all[:, ds(S * j + 128 * sc, 128)],
                    rhs=woutT_sb[j],
                    start=(j == 0),
                    stop=(j == NCH - 1),
                )
            m = P.tile([128, S], F16, name=f"M{sc}", tag=f"M{sc}")
            nc.vector.tensor_copy(m, ps)
            M_sb[sc] = m

        def out_early(h2, woutT_sb, bout_sb):
            rows = ds(h2 * HT, HT)
            out_ps = o_ps.tile([HT, D], F32, name="out_ps", tag="out_ps", bufs=1)
            nc.tensor.matmul(
                out_ps, lhsT=ones_sb[:, 0:HT], rhs=bout_sb, start=True, stop=False
            )
            for f in range(NCH, 2 * NCH):
                nc.tensor.matmul(
                    out_ps,
                    lhsT=inpT_sb[f - NCH][:, rows],
                    rhs=woutT_sb[f],
                    start=False,
                    stop=False,
                )
            out_ps_h[h2] = out_ps

        def zmm(tb, k, tl, rhs_slice):
            # one Z-matmul accumulating row (tb%HB)*4+tl of al_half[tb//HB]
            h2 = tb // HB
            t_loc = (tb % HB) * TBLK + tl
            n = blkmm.get(h2, 0)
            nc.tensor.matmul(
                al_half[h2],
                lhsT=Z[k][:, TH - 1 - t_loc : TH - 1 - t_loc + HT],
                rhs=rhs_slice,
                start=(n == 0),
                stop=(n == 8 * 16 - 1),
            )
            blkmm[h2] = n + 1

        align_sb = P.tile([TH, S], F32, name="align_sb", tag="align_sb")
        attn_sb = P.tile([TH, D], F32, name="attn_sb", tag="attn_sb")
        HB = NBLK // 2  # main-loop blocks per align half

        blk = {}  # tb -> dict of tiles

        def unit_add(tb, u, ui, eng, Bclamp=None):
            # pre-add for unit u; ACT units (ui = index into sum_t),
            # DVE units (ui = index into xm_t, with min clamp)
            b = blk[tb]
            k, tl = divmod(u, TBLK)
            t = tb * TBLK + tl
            if Bclamp is None:
                eng.tensor_scalar(
                    out=b["sum"][:, ds(ui * S, S)], in0=uh_sb[k],
                    scalar1=wqb_sb[k][:, t : t + 1], scalar2=None, op0=Add,
                )
            else:
                eng.tensor_scalar(
                    out=b["xm"][:, ds(ui * S, S)], in0=uh_sb[k],
                    scalar1=wqb_sb[k][:, t : t + 1], scalar2=Bclamp,
                    op0=Add, op1=Min,
                )

        def stage1(tb):
            # producers: tile allocation + all pre-adds / TS1s for block tb
            ND = ND_TB[tb]
            NA = 16 - ND
            b = blk[tb] = dict(
                sum=sums.tile([128, 16 * S], F16, name="sum_t", tag="sum_t"),
                tanh=tanhs.tile([128, 16 * S], F16, name="tanh_t", tag="tanh_t"),
            )
            if ND:
                b["xm"] = dvet.tile([128, ND_MAX * S], F16, name="xm_t", tag="xm_t")
                b["xc"] = dvet.tile([128, ND_MAX * S], F16, name="xc_t", tag="xc_t")
                b["g"] = dvet.tile([128, ND_MAX * S], F16, name="g_t", tag="g_t")
            if N_D13:
                b["p4"] = dvet.tile([128, N_D13 * S], F32, name="p4_t", tag="p4_t")
            for u in range(NPOOL):
                unit_add(tb, u, u, nc.gpsimd)
            for u in range(NPOOL, NA):
                unit_add(tb, u, u, nc.vector)
            for i, u in enumerate(range(NA, 16)):
                unit_add(tb, u, i, nc.vector, Bclamp=B13 if i < N_D13 else B7)

        def stage2(tb):
            # TS2 + custom ops over the packed DVE units of block tb
            ND = ND_TB[tb]
            N_D7 = ND - N_D13
            if not ND:
                return
            b = blk[tb]
            xm_t, xc_t, g_t = b["xm"], b["xc"], b["g"]
            if N_D13:
                sl = ds(0, N_D13 * S)
                nc.vector.tensor_scalar(
                    out=xc_t[:, sl], in0=xm_t[:, sl],
                    scalar1=float(-B13), scalar2=None, op0=Max,
                )
                nc.vector._custom_dve(
                    TANH13A, out=b["p4"], in0=xc_t[:, sl], in1=w13v,
                    s0=float(P13), s1=float(Q13), imm2=float(R13),
                )
                nc.vector._custom_dve(
                    TANH13B, out=g_t[:, sl], in0=b["p4"], in1=xc_t[:, sl],
                    s0=float(S13), s1=float(U13), imm2=float(A13),
                )
            if N_D7:
                sl = ds(N_D13 * S, N_D7 * S)
                nc.vector.tensor_scalar(
                    out=xc_t[:, sl], in0=xm_t[:, sl],
                    scalar1=float(-B7), scalar2=None, op0=Max,
                )
                nc.vector._custom_dve(
                    TANH7, out=g_t[:, sl], in0=xc_t[:, sl], in1=c1v,
                    s0=float(C7_), s1=float(C5_), imm2=float(C3_),
                )

        def act_stage(tb):
            # one tanh instruction per block: all producers ran >=1 block
            # ago (software pipelining), so the coarse dep costs nothing
            # and the per-instruction overhead is paid once.
            NA = 16 - ND_TB[tb]
            b = blk[tb]
            nc.scalar.activation(
                b["tanh"][:, ds(0, NA * S)], b["sum"][:, ds(0, NA * S)], Tanh
            )

        def pe_units(tb, units, tile_, base):
            for i, u in enumerate(units):
                k, tl = divmod(u, TBLK)
                zmm(tb, k, tl, tile_[:, ds((base + i) * S, S)])

        def pe_stage(tb, kw=9):
            ND = ND_TB[tb]
            NA = 16 - ND
            b = blk[tb]
            pe_units(tb, list(range(0, NA)), b["tanh"], 0)
            if ND:
                pe_units(tb, list(range(NA, 16)), b["g"], 0)
            keep_warm(kw)
            del blk[tb]

        def emit_block0():
            # chunk-at-a-time with just-in-time prologue phases (all-ACT)
            ND = ND_TB[0]
            NA = 16 - ND
            b = blk[0] = dict(
                sum=sums.tile([128, 16 * S], F16, name="sum_t", tag="sum_t"),
                tanh=tanhs.tile([128, 16 * S], F16, name="tanh_t", tag="tanh_t"),
            )
            if ND:
                b["xm"] = dvet.tile([128, ND_MAX * S], F16, name="xm_t", tag="xm_t")
                b["xc"] = dvet.tile([128, ND_MAX * S], F16, name="xc_t", tag="xc_t")
                b["g"] = dvet.tile([128, ND_MAX * S], F16, name="g_t", tag="g_t")
            if N_D13:
                b["p4"] = dvet.tile([128, N_D13 * S], F32, name="p4_t", tag="p4_t")
            # all adds + remaining prologue phases first: the uh_k matmuls
            # must not sit behind Z-matmuls (which wait on ACT) in the PE
            # FIFO. Block-0 adds are all-DVE: Pool's first work is block 1,
            # so the scheduler can't starve block-0's tanh stream with it.
            for k in range(NCH):
                for tl in range(TBLK):
                    u = k * TBLK + tl
                    if u < NA:
                        unit_add(0, u, u, nc.vector)
                    else:
                        unit_add(0, u, u - NA, nc.vector,
                                 Bclamp=B13 if u - NA < N_D13 else B7)
                if k + 1 < NCH:
                    prologue_phase([k + 1])
            for k in range(NCH):
                ka = [u for u in range(k * TBLK, (k + 1) * TBLK) if u < NA]
                if ka:
                    lo, hi = ka[0], ka[-1] + 1
                    nc.scalar.activation(
                        b["tanh"][:, ds(lo * S, (hi - lo) * S)],
                        b["sum"][:, ds(lo * S, (hi - lo) * S)], Tanh,
                    )
                    pe_units(0, ka, b["tanh"], lo)
            keep_warm(10)

        def finish_block0():
            ND = ND_TB[0]
            NA = 16 - ND
            if ND:
                stage2(0)
                pe_units(0, list(range(NA, 16)), blk[0]["g"], 0)
            del blk[0]

        # software-pipelined main loop: block n's producers (stage1) are
        # emitted ~2 blocks ahead of its DVE op chain (stage2), so the ACT
        # stream is never queued behind the custom-op work on the DVE FIFO.
        emit_block0()
        woutT_sb, bout_sb = load_epilogue_tensors()
        stage1(1)
        finish_block0()

        def last_block(tb):
            # final block of the run: all-ACT; emit tanh+matmuls in 4-unit
            # quarters so the align accumulation finishes (and the epilogue
            # starts) right after the last quarter instead of after one
            # monolithic 16-unit activation.
            b = blk[tb]
            for q in range(4):
                qs = ds(q * 4 * S, 4 * S)
                nc.scalar.activation(b["tanh"][:, qs], b["sum"][:, qs], Tanh)
                pe_units(tb, list(range(q * 4, q * 4 + 4)), b["tanh"], q * 4)
            keep_warm(4)
            del blk[tb]

        for tb in range(1, NBLK):
            if tb == NBLK - 1:
                stage2(tb)
                last_block(tb)
                epilogue_softmax_g(al_half[1], 1, 0, HT, kw=5)
                epilogue_close(1)
                epilogue_attn(1)
                break
            stage2(tb)
            act_stage(tb)
            pe_stage(tb)
            if tb + 1 < NBLK and (tb + 1) not in blk:
                stage1(tb + 1)
            if tb + 2 < NBLK:
                stage1(tb + 2)
            if 2 <= tb <= 5:
                emit_M_chunk(tb - 2, woutT_sb)
            if tb == HB - 4:
                out_early(0, woutT_sb, bout_sb)
            if tb == NBLK - 4:
                out_early(1, woutT_sb, bout_sb)
            # half-0 epilogue is emitted one block late so its cross-engine
            # chain doesn't head-of-line-block the next block's producers
            if tb == HB:
                epilogue_softmax_g(al_half[0], 0, 0, HT, kw=0)
                epilogue_close(0)
            if tb == HB + 1:
                epilogue_attn(0)
            if tb == NBLK - 1:
                epilogue_softmax(1, kw=7)
                epilogue_attn(1)


def get_nc():
    if "nc" not in _NC_CACHE:
        _NC_CACHE["nc"] = _build_nc()
    return _NC_CACHE["nc"]


def make_in_maps(inp, context, Wq, bq, Wc, v, Wout, bout):
    inp = np.asarray(inp, np.float32)
    context = np.asarray(context, np.float32)
    Wq = np.asarray(Wq, np.float32)
    bq = np.asarray(bq, np.float32)
    Wc = np.asarray(Wc, np.float32)
    v = np.asarray(v, np.float32)
    Wout = np.asarray(Wout, np.float32)
    bout = np.asarray(bout, np.float32)

    wqT = np.ascontiguousarray(Wq.T).astype(np.float16)
    wcT = np.ascontiguousarray(Wc.T).astype(np.float16)
    woutT = np.ascontiguousarray(Wout.T).astype(np.float16)
    in_maps = []
    for c in range(N_CORES):
        b, th = divmod(c, 2)
        in_maps.append(
            {
                "inpT": np.ascontiguousarray(
                    inp[b, th * TH : (th + 1) * TH].T
                ).astype(np.float16),
                "ctxT": np.ascontiguousarray(context[b].T).astype(np.float16),
                "wqT": wqT,
                "wcT": wcT,
                "woutT": woutT,
                "bq": bq,
                "v": v,
                "bout": bout,
            }
        )
    return in_maps


def run_on_device(in_maps, **kwargs):
    nc = get_nc()
    return run_bass_kernel_spmd(nc, in_maps, core_ids=list(range(N_CORES)), **kwargs)


def kernel(inp, context, Wq, bq, Wc, v, Wout, bout):
    in_maps = make_in_maps(inp, context, Wq, bq, Wc, v, Wout, bout)
    res = run_on_device(in_maps)
    attn = np.empty((B, T, D), np.float32)
    align = np.empty((B, T, S), np.float32)
    for c in range(N_CORES):
        b, th = divmod(c, 2)
        attn[b, th * TH : (th + 1) * TH] = res.results[c]["attn"]
        align[b, th * TH : (th + 1) * TH] = res.results[c]["align"]
    return attn, align
